# revision 5
# baseline (speedup 1.0000x reference)
"""Trainium2 Bass kernel for nn_PerformerSeperator (FAVOR+ transformer encoder).

Sharding: pure data-parallel over batch. B=32 is split 4-per-core across the
8 NeuronCores; every core runs the full 6-layer encoder on its shard with
replicated weights, so no collectives are needed.

Numerics: large matmuls run in fp32r (tf32; weights pre-rounded on host,
activations rounded by the producing ACT/DVE op). The per-head attention
matmuls (random-feature projections and the N=65 kvx/A contractions) run in
bf16. Everything else (layernorm, residual stream, FAVOR+ stabilizer
algebra) stays fp32. The eps/stabilizer algebra is restructured to be
layout-friendly while remaining exactly equivalent to the reference formula
(verified offline: fp32 impl matches reference to ~1e-7; tf32/bf16 rounding
contributes ~2e-4 absmax end to end).

Layouts: activations x live in SBUF as (T=4x128 partitions, DIM free) fp32
for the whole kernel. LN outputs are PE-transposed to D-major; q,k are
produced feature-major so per-head slices are partition ranges; v stays
t-major. The per-query stabilizer exp(q_sq + qmax) is applied as a
per-partition scalar in (T,.) layout against a partition-broadcast of the
eps * colsum(kvx) row, which keeps every op per-partition-scalar shaped.
"""
import numpy as np

B, F, T = 32, 256, 512
DIM, L, H, M = 512, 6, 8, 256
DH = DIM // H            # 64
FFD = 4 * DIM            # 2048
NM = 4
NCORES = 8
BL = B // NCORES         # 4 batch elements per core
DN = DH ** -0.25
EPS = 1e-4
DEN_EPS = float(1e-6 * M)   # 1e-6 / ratio^2, ratio = M**-0.5

_CACHE = {}


def _round_tf32(x):
    """Round fp32 array to tf32 (10-bit mantissa, RNE). Matches PE fp32r."""
    x = np.ascontiguousarray(x, np.float32)
    u = x.view(np.uint32).astype(np.uint64)
    bias = ((u >> 13) & 1) + 0xFFF
    u = (u + bias) & ~np.uint64(0x1FFF)
    return u.astype(np.uint32).view(np.float32)


def _build(flags, n_layers=L, n_b=BL):
    """Build the per-core Bass program. flags = (ubqk, ubv, ubo, ub1, ub2, ubm)."""
    import contextlib
    import concourse.bacc as bacc
    import concourse.tile as tile
    from concourse import mybir

    ubqk, ubv, ubo, ub1, ub2, ubm = flags
    DT = mybir.dt
    AFT = mybir.ActivationFunctionType
    ALU = mybir.AluOpType
    AXX = mybir.AxisListType.X
    F32, F32R, BF16 = DT.float32, DT.float32r, DT.bfloat16

    nc = bacc.Bacc("TRN2", target_bir_lowering=False, debug=False,
                   num_devices=NCORES)

    # ---------------- DRAM I/O ----------------
    mel_d = nc.dram_tensor("mel", [n_b, F, T], F32R, kind="ExternalInput").ap()
    pos_d = nc.dram_tensor("pos", [T, DIM], F32, kind="ExternalInput").ap()
    embw_d = nc.dram_tensor("embw", [F, DIM], F32R, kind="ExternalInput").ap()
    wqk_d = nc.dram_tensor("wqk", [n_layers, DIM, 2 * DIM], F32R, kind="ExternalInput").ap()
    bqk_d = nc.dram_tensor("bqk", [n_layers, 2 * DIM], F32, kind="ExternalInput").ap()
    wv_d = nc.dram_tensor("wv", [n_layers, DIM, DIM], F32R, kind="ExternalInput").ap()
    bv_d = nc.dram_tensor("bv", [n_layers, 1, DIM], F32R, kind="ExternalInput").ap()
    wtp_d = nc.dram_tensor("wtp", [n_layers, 128, M], BF16, kind="ExternalInput").ap()
    outw_d = nc.dram_tensor("outw", [n_layers, DIM, DIM], F32R, kind="ExternalInput").ap()
    outb_d = nc.dram_tensor("outb", [n_layers, 1, DIM], F32R, kind="ExternalInput").ap()
    w1_d = nc.dram_tensor("w1", [n_layers, DIM, FFD], F32R, kind="ExternalInput").ap()
    b1_d = nc.dram_tensor("b1", [n_layers, FFD], F32, kind="ExternalInput").ap()
    w2_d = nc.dram_tensor("w2", [n_layers, FFD, DIM], F32R, kind="ExternalInput").ap()
    b2_d = nc.dram_tensor("b2", [n_layers, 1, DIM], F32R, kind="ExternalInput").ap()
    maskw_d = nc.dram_tensor("maskw", [DIM, NM], F32R, kind="ExternalInput").ap()
    maskb_d = nc.dram_tensor("maskb", [NM, 1], F32, kind="ExternalInput").ap()
    hones_d = nc.dram_tensor("hones", [128, 4, H], BF16, kind="ExternalInput").ap()
    ident_d = nc.dram_tensor("ident", [128, 128], F32, kind="ExternalInput").ap()
    identr_d = nc.dram_tensor("identr", [128, 128], F32R, kind="ExternalInput").ap()
    onesr_d = nc.dram_tensor("onesr", [1, 128], F32R, kind="ExternalInput").ap()
    ones32_d = nc.dram_tensor("ones32", [1, 128], F32, kind="ExternalInput").ap()
    out_d = nc.dram_tensor("masks", [n_b, NM, T], F32, kind="ExternalOutput").ap()

    with tile.TileContext(nc) as tc:
        with contextlib.ExitStack() as stack:
            consts = stack.enter_context(tc.tile_pool(name="consts", bufs=1))
            xpool = stack.enter_context(tc.tile_pool(name="xpool", bufs=1))
            wpool = stack.enter_context(tc.tile_pool(name="wpool", bufs=1))
            ws = stack.enter_context(tc.tile_pool(name="ws", bufs=1))
            st = stack.enter_context(tc.tile_pool(name="st", bufs=2))
            psmm = stack.enter_context(
                tc.tile_pool(name="psmm", bufs=2, space="PSUM"))

            # ---------------- constants ----------------
            ident = consts.tile([128, 128], F32)
            nc.sync.dma_start(ident[:], ident_d[:])
            identr = consts.tile([128, 128], F32R)
            nc.sync.dma_start(identr[:], identr_d[:])
            onesr = consts.tile([1, 128], F32R)
            nc.sync.dma_start(onesr[:], onesr_d[:])
            ones32 = consts.tile([1, 128], F32)
            nc.sync.dma_start(ones32[:], ones32_d[:])
            hones = consts.tile([128, 4, H], BF16)
            nc.sync.dma_start(hones[:], hones_d[:])
            maskw = consts.tile([128, 4, NM], F32R)
            nc.sync.dma_start(maskw[:], maskw_d.rearrange("(c p) n -> p c n", p=128))
            maskb = consts.tile([NM, 1], F32)
            nc.sync.dma_start(maskb[:], maskb_d[:])
            onescol_bf = consts.tile([128, 1], BF16)
            nc.gpsimd.memset(onescol_bf[:], 1.0)
            lneps = consts.tile([128, 1], F32)
            nc.gpsimd.memset(lneps[:], 1e-5)

            # persistent activations: x[b] = (128 t-part, 4 t-chunks, DIM)
            xs = [xpool.tile([128, 4, DIM], F32, name=f"x{b}") for b in range(n_b)]

            # ---------------- embedding ----------------
            # (reuses later-phase slots: embw->wv, pos->o_all, mel->hT)
            embw = wpool.tile([128, 2, DIM], F32R, tag="wv", name="embw")
            nc.sync.dma_start(embw[:], embw_d.rearrange("(c p) d -> p c d", p=128))
            pos = ws.tile([128, 4, DIM], F32, tag="o_all", bufs=1, name="pos")
            nc.sync.dma_start(pos[:], pos_d.rearrange("(c p) d -> p c d", p=128))
            for b in range(n_b):
                mel_sb = ws.tile([128, 2, T], F32R, tag="hT", bufs=2,
                                 name=f"mel{b}")
                nc.sync.dma_start(
                    mel_sb[:], mel_d[b].rearrange("(c p) t -> p c t", p=128))
                for tcn in range(4):
                    ps = psmm.tile([128, DIM], F32, tag="mm",
                                   name=f"emb{nc.next_id()}")
                    for k in range(2):
                        nc.tensor.matmul(
                            ps[:], mel_sb[:, k, tcn * 128:(tcn + 1) * 128],
                            embw[:, k], start=(k == 0), stop=(k == 1))
                    nc.vector.tensor_add(xs[b][:, tcn], ps[:], pos[:, tcn])

            # layer weights (single-buffered; loads overlap prior-layer compute)
            def load_layer_weights(l):
                wqk = wpool.tile([128, 4, 2 * DIM], F32R, tag="wqk", name=f"wqk{l}")
                nc.sync.dma_start(wqk[:], wqk_d[l].rearrange("(c p) n -> p c n", p=128))
                wv = wpool.tile([128, 4, DIM], F32R, tag="wv", name=f"wv{l}")
                nc.sync.dma_start(wv[:], wv_d[l].rearrange("(c p) n -> p c n", p=128))
                wtp = wpool.tile([128, M], BF16, tag="wtp", name=f"wtp{l}")
                nc.sync.dma_start(wtp[:], wtp_d[l])
                outw = wpool.tile([128, 4, DIM], F32R, tag="outw", name=f"outw{l}")
                nc.sync.dma_start(outw[:], outw_d[l].rearrange("(c p) n -> p c n", p=128))
                w1 = wpool.tile([128, 4, FFD], F32R, tag="w1", name=f"w1{l}")
                nc.sync.dma_start(w1[:], w1_d[l].rearrange("(c p) n -> p c n", p=128))
                w2 = wpool.tile([128, 16, DIM], F32R, tag="w2", name=f"w2{l}")
                nc.sync.dma_start(w2[:], w2_d[l].rearrange("(c p) n -> p c n", p=128))
                d = {"wqk": wqk, "wv": wv, "wtp": wtp, "outw": outw,
                     "w1": w1, "w2": w2}
                if ubqk:
                    bqk = wpool.tile([128, 8], F32, tag="bqk", name=f"bqk{l}")
                    nc.sync.dma_start(bqk[:], bqk_d[l].rearrange("(c p) -> p c", p=128))
                    d["bqk"] = bqk
                if ubv:
                    bv = wpool.tile([1, DIM], F32R, tag="bv", name=f"bv{l}")
                    nc.sync.dma_start(bv[:], bv_d[l])
                    d["bv"] = bv
                if ubo:
                    outb = wpool.tile([1, DIM], F32R, tag="outb", name=f"outb{l}")
                    nc.sync.dma_start(outb[:], outb_d[l])
                    d["outb"] = outb
                if ub1:
                    b1 = wpool.tile([128, 16], F32, tag="b1", name=f"b1{l}")
                    nc.sync.dma_start(b1[:], b1_d[l].rearrange("(c p) -> p c", p=128))
                    d["b1"] = b1
                if ub2:
                    b2 = wpool.tile([1, DIM], F32R, tag="b2", name=f"b2{l}")
                    nc.sync.dma_start(b2[:], b2_d[l])
                    d["b2"] = b2
                return d

            def layernorm_transposed(xb):
                """LN over the free dim of each (128,512) chunk of xb; output
                PE-transposed into a (128, 4, T) fp32r tile (D-major).
                Uses var = E[x^2] - mean^2 (matches reference to ~1e-6)."""
                hT = ws.tile([128, 4, T], F32R, tag="hT", bufs=2,
                             name=f"hT{nc.next_id()}")
                for tcn in range(4):
                    xtc = xb[:, tcn]
                    dump = ws.tile([128, T], F32, tag="dump", bufs=1,
                                   name=f"dmp{nc.next_id()}")
                    ssq = st.tile([128, 1], F32, tag="ssq", name=f"ssq{nc.next_id()}")
                    nc.vector.scalar_tensor_tensor(
                        dump[:], xtc, 1.0, xtc, op0=ALU.mult, op1=ALU.mult,
                        accum_out=ssq[:])
                    s1 = st.tile([128, 1], F32, tag="s1", name=f"s1{nc.next_id()}")
                    nc.vector.reduce_sum(s1[:], xtc, axis=AXX)
                    mus = st.tile([128, 1], F32, tag="mus", name=f"mus{nc.next_id()}")
                    nc.scalar.mul(mus[:], s1[:], 1.0 / DIM)
                    musq = st.tile([128, 1], F32, tag="musq", name=f"msq{nc.next_id()}")
                    nc.scalar.square(musq[:], mus[:])
                    var = st.tile([128, 1], F32, tag="var", name=f"var{nc.next_id()}")
                    nc.vector.scalar_tensor_tensor(
                        var[:], ssq[:], 1.0 / DIM, musq[:],
                        op0=ALU.mult, op1=ALU.subtract)
                    std = st.tile([128, 1], F32, tag="std", name=f"std{nc.next_id()}")
                    nc.scalar.activation(std[:], var[:], AFT.Sqrt, bias=lneps[:])
                    rstd = st.tile([128, 1], F32, tag="rstd", name=f"rs{nc.next_id()}")
                    nc.vector.reciprocal(rstd[:], std[:])
                    h = ws.tile([128, T], F32R, tag="h", bufs=2,
                                name=f"h{nc.next_id()}")
                    nc.vector.tensor_scalar(h[:], xtc, mus[:], rstd[:],
                                            op0=ALU.subtract, op1=ALU.mult)
                    tp = psmm.tile([128, 4, 128], F32R, tag="mm",
                                   name=f"tp{nc.next_id()}")
                    for i in range(4):
                        nc.tensor.transpose(tp[:, i], h[:, i * 128:(i + 1) * 128],
                                            identr[:])
                    nc.vector.tensor_copy(
                        hT[:, :, tcn * 128:(tcn + 1) * 128], tp[:])
                return hT

            def transpose_o(o_all):
                oT = ws.tile([128, 4, T], F32R, tag="oT", bufs=1,
                             name=f"oT{nc.next_id()}")
                for tcn in range(4):
                    tp = psmm.tile([128, 4, 128], F32R, tag="mm",
                                   name=f"otp{nc.next_id()}")
                    for i in range(4):
                        nc.tensor.transpose(
                            tp[:, i], o_all[:, tcn, i * 128:(i + 1) * 128],
                            identr[:])
                    nc.vector.tensor_copy(
                        oT[:, :, tcn * 128:(tcn + 1) * 128], tp[:])
                return oT

            def attention(l, wts, b, psatt):
                hT = layernorm_transposed(xs[b])
                wqk, wv, wtp = wts["wqk"], wts["wv"], wts["wtp"]

                # q^T,k^T feature-major in bf16: qkT[:, fc] = (128 feat, T)
                qkT = ws.tile([128, 8, T], BF16, tag="qkT", bufs=1,
                              name=f"qkT{nc.next_id()}")
                for fc in range(8):
                    ps = psmm.tile([128, T], F32, tag="mm", name=f"qk{nc.next_id()}")
                    for k in range(4):
                        nc.tensor.matmul(
                            ps[:], wqk[:, k, fc * 128:(fc + 1) * 128], hT[:, k],
                            start=(k == 0), stop=(k == 3))
                    if ubqk:
                        nc.scalar.activation(qkT[:, fc], ps[:], AFT.Identity,
                                             bias=wts["bqk"][:, fc:fc + 1])
                    else:
                        nc.scalar.copy(qkT[:, fc], ps[:])

                # v (t-major) into strided bf16 vx with ones columns
                vx = ws.tile([128, 4, H, 65], BF16, tag="vx", bufs=1,
                             name=f"vx{nc.next_id()}")
                for tcn in range(4):
                    nc.vector.memset(vx[:, tcn, :, 64:65], 1.0)
                    ps = psmm.tile([128, DIM], F32, tag="mm", name=f"v{nc.next_id()}")
                    for k in range(4):
                        nc.tensor.matmul(
                            ps[:], hT[:, k, tcn * 128:(tcn + 1) * 128], wv[:, k],
                            start=(k == 0), stop=(k == 3 and not ubv))
                    if ubv:
                        nc.tensor.matmul(ps[:], onesr[:], wts["bv"][:],
                                         start=False, stop=True)
                    nc.vector.tensor_copy(
                        vx[:, tcn, :, 0:64],
                        ps.rearrange("p (h d) -> p h d", d=64))

                # q_sq/k_sq: bf16 squares + blockdiag-halfones matmuls
                # qksq[:, tc, 0:8] = 0.5*sum qd^2 per head ; [:, tc, 8:16] = k
                qksq = ws.tile([128, 4, 16], F32, tag="qksq", bufs=1,
                               name=f"qksq{nc.next_id()}")
                for half in range(2):          # 0: q (chunks 0-3), 1: k (4-7)
                    sqs = []
                    for k in range(4):
                        sq = ws.tile([128, T], BF16, tag="sq", bufs=4,
                                     name=f"sq{nc.next_id()}")
                        qk = qkT[:, 4 * half + k]
                        nc.vector.scalar_tensor_tensor(
                            sq[:], qk, 1.0, qk, op0=ALU.mult, op1=ALU.mult)
                        sqs.append(sq)
                    for tcn in range(4):
                        pst = psatt.tile([128, H], F32, tag="tiny",
                                         name=f"sqp{nc.next_id()}")
                        for k in range(4):
                            nc.tensor.matmul(
                                pst[:], sqs[k][:, tcn * 128:(tcn + 1) * 128],
                                hones[:, k], start=(k == 0), stop=(k == 3))
                        nc.scalar.copy(qksq[:, tcn, 8 * half:8 * half + 8], pst[:])

                o_all = ws.tile([128, 4, DIM], F32R, tag="o_all", bufs=1,
                                name=f"o{nc.next_id()}")

                for h in range(H):
                    dc, off = divmod(h, 2)
                    o0, o1 = off * 64, off * 64 + 64
                    tpos = (64, 0) if off else None
                    kc = 4 + dc

                    # kp (T,M) in psum; global kmax
                    kps = []
                    for tcn in range(4):
                        ps = psatt.tile([128, M], F32, tag="h256", bufs=4,
                                        name=f"kp{nc.next_id()}")
                        nc.tensor.matmul(
                            ps[:], qkT[o0:o1, kc, tcn * 128:(tcn + 1) * 128],
                            wtp[o0:o1], start=True, stop=True,
                            tile_position=tpos)
                        kps.append(ps)
                    kmx = st.tile([128, 1], F32, tag="kmx", name=f"km{nc.next_id()}")
                    nc.vector.reduce_max(kmx[:], kps[0][:], axis=AXX)
                    for tcn in range(1, 4):
                        km2 = st.tile([128, 1], F32, tag="kmx2",
                                      name=f"km2{nc.next_id()}")
                        nc.vector.reduce_max(km2[:], kps[tcn][:], axis=AXX)
                        nc.vector.tensor_max(kmx[:], kmx[:], km2[:])
                    kmt = psatt.tile([1, 128], F32, tag="tiny",
                                     name=f"kmt{nc.next_id()}")
                    nc.tensor.transpose(kmt[:], kmx[:], ident[:])
                    kms = st.tile([1, 1], F32, tag="kms", name=f"kms{nc.next_id()}")
                    nc.vector.reduce_max(kms[:], kmt[:], axis=AXX)
                    kbc_ps = psatt.tile([128, 1], F32, tag="tiny",
                                        name=f"kbc{nc.next_id()}")
                    nc.tensor.matmul(kbc_ps[:], ones32[:], kms[:],
                                     start=True, stop=True)
                    kmbc = st.tile([128, 1], F32, tag="kmbc",
                                   name=f"kmbc{nc.next_id()}")
                    nc.scalar.copy(kmbc[:], kbc_ps[:])

                    # k_phi = exp(kp - ksq - kmax) + EPS  (bf16)
                    kphi = ws.tile([128, 4, M], BF16, tag="kphi", bufs=2,
                                   name=f"kph{nc.next_id()}")
                    for tcn in range(4):
                        kb = st.tile([128, 1], F32, tag="kb",
                                     name=f"kb{nc.next_id()}")
                        nc.vector.scalar_tensor_tensor(
                            kb[:], qksq[:, tcn, 8 + h:9 + h], -1.0, kmbc[:],
                            op0=ALU.mult, op1=ALU.subtract)
                        nc.scalar.activation(kphi[:, tcn], kps[tcn][:], AFT.Exp,
                                             bias=kb[:])
                        nc.vector.tensor_scalar_add(kphi[:, tcn], kphi[:, tcn],
                                                    EPS)

                    # kvx (M=2x128 part, 65) accumulated over t
                    kvx_ps = psatt.tile([128, 2, 65], F32, tag="tiny",
                                        name=f"kvp{nc.next_id()}")
                    for mh in range(2):
                        for tcn in range(4):
                            nc.tensor.matmul(
                                kvx_ps[:, mh],
                                kphi[:, tcn, mh * 128:(mh + 1) * 128],
                                vx[:, tcn, h], start=(tcn == 0), stop=(tcn == 3))
                    kvx = ws.tile([128, 2, 65], BF16, tag="kvx", bufs=2,
                                  name=f"kvs{nc.next_id()}")
                    nc.scalar.copy(kvx[:], kvx_ps[:])

                    # row_s = EPS * colsum(kvx); [64] += DEN_EPS; bcast to 128p
                    rs_ps = psatt.tile([1, 65], F32, tag="tiny",
                                       name=f"rsp{nc.next_id()}")
                    for mh in range(2):
                        nc.tensor.matmul(rs_ps[:], onescol_bf[:], kvx[:, mh],
                                         start=(mh == 0), stop=(mh == 1))
                    rs = st.tile([1, 65], F32, tag="rs", name=f"rss{nc.next_id()}")
                    nc.scalar.mul(rs[:], rs_ps[:], EPS)
                    nc.vector.tensor_scalar_add(rs[0:1, 64:65], rs[0:1, 64:65],
                                                DEN_EPS)
                    sb_ps = psatt.tile([128, 65], F32, tag="tiny",
                                       name=f"sbp{nc.next_id()}")
                    nc.tensor.matmul(sb_ps[:], ones32[:], rs[:],
                                     start=True, stop=True)
                    s_b = st.tile([128, 65], F32, tag="s_b",
                                  name=f"sbb{nc.next_id()}")
                    nc.scalar.copy(s_b[:], sb_ps[:])

                    # qmax per query from qp (T,M)
                    qmax = st.tile([128, 4], F32, tag="qmax",
                                   name=f"qm{nc.next_id()}")
                    for tcn in range(4):
                        ps = psatt.tile([128, M], F32, tag="h256", bufs=4,
                                        name=f"qp{nc.next_id()}")
                        nc.tensor.matmul(
                            ps[:], qkT[o0:o1, dc, tcn * 128:(tcn + 1) * 128],
                            wtp[o0:o1], start=True, stop=True,
                            tile_position=tpos)
                        nc.vector.reduce_max(qmax[:, tcn:tcn + 1], ps[:], axis=AXX)

                    # e^{qp^T} (M-major, bf16)
                    eqpT = ws.tile([128, 2, T], BF16, tag="eqpT", bufs=2,
                                   name=f"eq{nc.next_id()}")
                    for mh in range(2):
                        ps = psmm.tile([128, T], F32, tag="mm",
                                       name=f"qpT{nc.next_id()}")
                        nc.tensor.matmul(
                            ps[:], wtp[o0:o1, mh * 128:(mh + 1) * 128],
                            qkT[o0:o1, dc], start=True, stop=True,
                            tile_position=tpos)
                        nc.scalar.activation(eqpT[:, mh], ps[:], AFT.Exp)

                    # A = eqpT^T @ kvx  (T,65) per t-chunk, packed in one bank
                    A_ps = psatt.tile([128, 4, 65], F32, tag="tiny",
                                      name=f"A{nc.next_id()}")
                    for tcn in range(4):
                        for mh in range(2):
                            nc.tensor.matmul(
                                A_ps[:, tcn],
                                eqpT[:, mh, tcn * 128:(tcn + 1) * 128],
                                kvx[:, mh], start=(mh == 0), stop=(mh == 1))

                    # o_ext = A + e^{qsq+qmax} * row_s ; divide ; -> o_all
                    for tcn in range(4):
                        g = st.tile([128, 1], F32, tag="g", name=f"g{nc.next_id()}")
                        nc.scalar.activation(g[:], qksq[:, tcn, h:h + 1], AFT.Exp,
                                             bias=qmax[:, tcn:tcn + 1])
                        oe = st.tile([128, 65], F32, tag="oe",
                                     name=f"oe{nc.next_id()}")
                        nc.vector.scalar_tensor_tensor(
                            oe[:], s_b[:], g[:], A_ps[:, tcn],
                            op0=ALU.mult, op1=ALU.add)
                        rec = st.tile([128, 1], F32, tag="rec",
                                      name=f"rc{nc.next_id()}")
                        nc.vector.reciprocal(rec[:], oe[:, 64:65])
                        nc.vector.tensor_scalar_mul(
                            o_all[:, tcn, h * 64:(h + 1) * 64], oe[:, 0:64],
                            rec[:])

                # out-proj + residual
                oT = transpose_o(o_all)
                outw = wts["outw"]
                for tcn in range(4):
                    ps = psmm.tile([128, DIM], F32, tag="mm",
                                   name=f"op{nc.next_id()}")
                    for k in range(4):
                        nc.tensor.matmul(
                            ps[:], oT[:, k, tcn * 128:(tcn + 1) * 128],
                            outw[:, k], start=(k == 0),
                            stop=(k == 3 and not ubo))
                    if ubo:
                        nc.tensor.matmul(ps[:], onesr[:], wts["outb"][:],
                                         start=False, stop=True)
                    nc.vector.tensor_add(xs[b][:, tcn], ps[:], xs[b][:, tcn])

            def ffn(l, wts, b, psffn):
                h2T = layernorm_transposed(xs[b])
                w1, w2 = wts["w1"], wts["w2"]
                accs = [psffn.tile([128, DIM], F32, tag="acc",
                                   name=f"fa{nc.next_id()}") for _ in range(4)]
                for fc in range(16):
                    ps = psmm.tile([128, T], F32, tag="mm",
                                   name=f"g1{nc.next_id()}")
                    for k in range(4):
                        nc.tensor.matmul(
                            ps[:], w1[:, k, fc * 128:(fc + 1) * 128], h2T[:, k],
                            start=(k == 0), stop=(k == 3))
                    gt = ws.tile([128, T], F32R, tag="gt", bufs=2,
                                 name=f"gt{nc.next_id()}")
                    if ub1:
                        nc.scalar.activation(gt[:], ps[:], AFT.Gelu_apprx_tanh,
                                             bias=wts["b1"][:, fc:fc + 1])
                    else:
                        nc.scalar.activation(gt[:], ps[:], AFT.Gelu_apprx_tanh)
                    for tcn in range(4):
                        nc.tensor.matmul(
                            accs[tcn][:], gt[:, tcn * 128:(tcn + 1) * 128],
                            w2[:, fc], start=(fc == 0),
                            stop=(fc == 15 and not ub2))
                for tcn in range(4):
                    if ub2:
                        nc.tensor.matmul(accs[tcn][:], onesr[:], wts["b2"][:],
                                         start=False, stop=True)
                    nc.vector.tensor_add(xs[b][:, tcn], accs[tcn][:],
                                         xs[b][:, tcn])

            # ---------------- layers ----------------
            for l in range(n_layers):
                wts = load_layer_weights(l)
                with tc.tile_pool(name=f"psatt{l}", bufs=2,
                                  space="PSUM") as psatt:
                    for b in range(n_b):
                        attention(l, wts, b, psatt)
                with tc.tile_pool(name=f"psffn{l}", bufs=4,
                                  space="PSUM") as psffn:
                    for b in range(n_b):
                        ffn(l, wts, b, psffn)

            # ---------------- final masks ----------------
            for b in range(n_b):
                xT = ws.tile([128, 4, T], F32R, tag="hT", bufs=2,
                             name=f"xT{nc.next_id()}")
                for tcn in range(4):
                    tp = psmm.tile([128, 4, 128], F32, tag="mm",
                                   name=f"xtp{nc.next_id()}")
                    for i in range(4):
                        nc.tensor.transpose(
                            tp[:, i], xs[b][:, tcn, i * 128:(i + 1) * 128],
                            ident[:])
                    nc.vector.tensor_copy(
                        xT[:, :, tcn * 128:(tcn + 1) * 128], tp[:])
                yps = psmm.tile([128, T], F32, tag="mm", name=f"y{nc.next_id()}")
                for k in range(4):
                    nc.tensor.matmul(yps[0:NM], maskw[:, k], xT[:, k],
                                     start=(k == 0), stop=(k == 3))
                ysb = ws.tile([NM, T], F32, tag="ysb", bufs=1,
                              name=f"ys{nc.next_id()}")
                if ubm:
                    nc.scalar.activation(ysb[:], yps[0:NM], AFT.Sigmoid,
                                         bias=maskb[:])
                else:
                    nc.scalar.activation(ysb[:], yps[0:NM], AFT.Sigmoid)
                nc.sync.dma_start(out_d[b], ysb[:])

    nc.compile()
    return nc


def _prep_inputs(inputs, n_layers=L, n_b_total=B):
    """Host-side weight folding/rounding. Returns (per-core in_maps, flags)."""
    import ml_dtypes
    bf16 = ml_dtypes.bfloat16
    f32 = lambda a: np.ascontiguousarray(a, np.float32)
    mel = f32(inputs["mel"])[:n_b_total]
    to_emb_w = f32(inputs["to_emb_w"])
    to_emb_b = f32(inputs["to_emb_b"])
    pos_emb = f32(inputs["pos_emb"])
    proj = f32(inputs["proj"])
    qkv_w = f32(inputs["qkv_w"])
    qkv_b = f32(inputs["qkv_b"])
    out_w = f32(inputs["out_w"])
    out_b = f32(inputs["out_b"])
    ln1_g = f32(inputs["ln1_g"])
    ln1_b = f32(inputs["ln1_b"])
    ln2_g = f32(inputs["ln2_g"])
    ln2_b = f32(inputs["ln2_b"])
    ff1_w = f32(inputs["ff1_w"])
    ff1_b = f32(inputs["ff1_b"])
    ff2_w = f32(inputs["ff2_w"])
    ff2_b = f32(inputs["ff2_b"])
    mask_w = f32(inputs["mask_w"])
    mask_b = f32(inputs["mask_b"])

    nl = n_layers
    Wfold = qkv_w[:nl] * ln1_g[:nl][:, :, None]          # (L, D, 3D)
    bias_qkv = np.einsum("ld,ldn->ln", ln1_b[:nl], qkv_w[:nl]) + qkv_b[:nl]
    wq = Wfold[:, :, :DIM] * DN
    wk = Wfold[:, :, DIM:2 * DIM] * DN
    wv = Wfold[:, :, 2 * DIM:]
    bqk = np.concatenate([bias_qkv[:, :DIM] * DN,
                          bias_qkv[:, DIM:2 * DIM] * DN], axis=1)  # (L, 1024)
    bv = bias_qkv[:, None, 2 * DIM:]                     # (L, 1, D)
    W1fold = ff1_w[:nl] * ln2_g[:nl][:, :, None]
    b1 = np.einsum("ld,ldn->ln", ln2_b[:nl], ff1_w[:nl]) + ff1_b[:nl]
    wtpT = np.transpose(proj[:nl], (0, 2, 1))            # (L, DH, M)
    wtp = np.concatenate([wtpT, wtpT], axis=1)           # (L, 128, M) doubled

    hones = np.zeros((128, 4, H), np.float32)
    for d in range(DIM):
        hones[d % 128, d // 128, d // DH] = 0.5
    ident = np.eye(128, dtype=np.float32)

    common = {
        "pos": f32(pos_emb[0, :T] + to_emb_b),
        "embw": _round_tf32(to_emb_w),
        "wqk": _round_tf32(np.concatenate([wq, wk], axis=2)),
        "bqk": f32(bqk),
        "wv": _round_tf32(wv),
        "bv": _round_tf32(bv),
        "wtp": np.ascontiguousarray(wtp.astype(bf16)),
        "outw": _round_tf32(out_w[:nl]),
        "outb": _round_tf32(out_b[:nl][:, None, :]),
        "w1": _round_tf32(W1fold),
        "b1": f32(b1),
        "w2": _round_tf32(ff2_w[:nl]),
        "b2": _round_tf32(ff2_b[:nl][:, None, :]),
        "maskw": _round_tf32(mask_w),
        "maskb": f32(mask_b[:, None]),
        "hones": np.ascontiguousarray(hones.astype(bf16)),
        "ident": ident,
        "identr": ident.copy(),
        "onesr": np.ones((1, 128), np.float32),
        "ones32": np.ones((1, 128), np.float32),
    }
    flags = (bool(np.any(bqk)), bool(np.any(bv)),
             bool(np.any(out_b[:nl])), bool(np.any(b1)),
             bool(np.any(ff2_b[:nl])), bool(np.any(mask_b)))

    mel_r = _round_tf32(mel)
    n_cores_used = max(1, n_b_total // BL)
    in_maps = []
    for c in range(n_cores_used):
        m = dict(common)
        m["mel"] = mel_r[c * BL:(c + 1) * BL]
        in_maps.append(m)
    return in_maps, flags


def kernel(**inputs):
    from concourse.bass_utils import run_bass_kernel_spmd

    in_maps, flags = _prep_inputs(inputs)
    key = ("full", flags)
    if key not in _CACHE:
        _CACHE[key] = _build(flags)
    nc = _CACHE[key]
    res = run_bass_kernel_spmd(nc, in_maps, list(range(NCORES)))
    out = np.concatenate([res.results[c]["masks"] for c in range(NCORES)],
                         axis=0)
    return np.ascontiguousarray(out, np.float32)


# revision 11
# speedup vs baseline: 1.2636x; 1.2636x over previous
"""Trainium2 Bass kernel for nn_PerformerSeperator (FAVOR+ transformer encoder).

Sharding: pure data-parallel over batch. B=32 is split 4-per-core across the
8 NeuronCores; every core runs the full 6-layer encoder on its shard with
replicated weights, so no collectives are needed.

Numerics: large matmuls run in fp32r (tf32; weights pre-rounded on host,
activations rounded by the producing ACT/DVE op). The per-head attention
matmuls (random-feature projections and the N=65 kvx/A contractions) run in
bf16. Everything else (layernorm, residual stream, FAVOR+ stabilizer
algebra) stays fp32. The eps/stabilizer algebra is restructured to be
layout-friendly while remaining exactly equivalent to the reference formula
(verified offline: fp32 impl matches reference to ~1e-7; tf32/bf16 rounding
contributes ~2e-4 absmax end to end).

Layouts: activations x live in SBUF as (T=4x128 partitions, DIM free) fp32
for the whole kernel. LN outputs are PE-transposed to D-major; q,k are
produced feature-major so per-head slices are partition ranges; v stays
t-major. The per-query stabilizer exp(q_sq + qmax) is applied as a
per-partition scalar in (T,.) layout against a partition-broadcast of the
eps * colsum(kvx) row, which keeps every op per-partition-scalar shaped.
"""
import os
import numpy as np

USE_PAR = os.environ.get("K_PAR", "1") == "1"
USE_PAIR = os.environ.get("K_PAIR", "1") == "1"
USE_BN = os.environ.get("K_BN", "1") == "1"

B, F, T = 32, 256, 512
DIM, L, H, M = 512, 6, 8, 256
DH = DIM // H            # 64
FFD = 4 * DIM            # 2048
NM = 4
NCORES = 8
BL = B // NCORES         # 4 batch elements per core
DN = DH ** -0.25
EPS = 1e-4
DEN_EPS = float(1e-6 * M)   # 1e-6 / ratio^2, ratio = M**-0.5

_CACHE = {}


def _round_tf32(x):
    """Round fp32 array to tf32 (10-bit mantissa, RNE). Matches PE fp32r."""
    x = np.ascontiguousarray(x, np.float32)
    u = x.view(np.uint32).astype(np.uint64)
    bias = ((u >> 13) & 1) + 0xFFF
    u = (u + bias) & ~np.uint64(0x1FFF)
    return u.astype(np.uint32).view(np.float32)


def _build(flags, n_layers=L, n_b=BL):
    """Build the per-core Bass program. flags = (ubqk, ubv, ubo, ub1, ub2, ubm)."""
    import contextlib
    import concourse.bacc as bacc
    import concourse.tile as tile
    from concourse import bass_isa, mybir

    ubqk, ubv, ubo, ub1, ub2, ubm = flags
    DT = mybir.dt
    AFT = mybir.ActivationFunctionType
    ALU = mybir.AluOpType
    AXX = mybir.AxisListType.X
    F32, F32R, BF16 = DT.float32, DT.float32r, DT.bfloat16

    nc = bacc.Bacc("TRN2", target_bir_lowering=False, debug=False,
                   num_devices=NCORES)

    # ---------------- DRAM I/O ----------------
    mel_d = nc.dram_tensor("mel", [n_b, F, T], F32R, kind="ExternalInput").ap()
    pos_d = nc.dram_tensor("pos", [T, DIM], F32, kind="ExternalInput").ap()
    embw_d = nc.dram_tensor("embw", [F, DIM], F32R, kind="ExternalInput").ap()
    wqk_d = nc.dram_tensor("wqk", [n_layers, DIM, 2 * DIM], F32R, kind="ExternalInput").ap()
    bqk_d = nc.dram_tensor("bqk", [n_layers, 2 * DIM], F32, kind="ExternalInput").ap()
    wv_d = nc.dram_tensor("wv", [n_layers, DIM, DIM], F32R, kind="ExternalInput").ap()
    bv_d = nc.dram_tensor("bv", [n_layers, 1, DIM], F32R, kind="ExternalInput").ap()
    wtp_d = nc.dram_tensor("wtp", [n_layers, 128, M], BF16, kind="ExternalInput").ap()
    outw_d = nc.dram_tensor("outw", [n_layers, DIM, DIM], F32R, kind="ExternalInput").ap()
    outb_d = nc.dram_tensor("outb", [n_layers, 1, DIM], F32R, kind="ExternalInput").ap()
    w1_d = nc.dram_tensor("w1", [n_layers, DIM, FFD], F32R, kind="ExternalInput").ap()
    b1_d = nc.dram_tensor("b1", [n_layers, FFD], F32, kind="ExternalInput").ap()
    w2_d = nc.dram_tensor("w2", [n_layers, FFD, DIM], F32R, kind="ExternalInput").ap()
    b2_d = nc.dram_tensor("b2", [n_layers, 1, DIM], F32R, kind="ExternalInput").ap()
    maskw_d = nc.dram_tensor("maskw", [DIM, NM], F32R, kind="ExternalInput").ap()
    maskb_d = nc.dram_tensor("maskb", [NM, 1], F32, kind="ExternalInput").ap()
    hones_d = nc.dram_tensor("hones", [128, 4, H], BF16, kind="ExternalInput").ap()
    ident_d = nc.dram_tensor("ident", [128, 128], F32, kind="ExternalInput").ap()
    identr_d = nc.dram_tensor("identr", [128, 128], F32R, kind="ExternalInput").ap()
    onesr_d = nc.dram_tensor("onesr", [1, 128], F32R, kind="ExternalInput").ap()
    ones32_d = nc.dram_tensor("ones32", [1, 128], F32, kind="ExternalInput").ap()
    out_d = nc.dram_tensor("masks", [n_b, NM, T], F32, kind="ExternalOutput").ap()

    with tile.TileContext(nc) as tc:
        with contextlib.ExitStack() as stack:
            consts = stack.enter_context(tc.tile_pool(name="consts", bufs=1))
            xpool = stack.enter_context(tc.tile_pool(name="xpool", bufs=1))
            wpool = stack.enter_context(tc.tile_pool(name="wpool", bufs=1))
            ws = stack.enter_context(tc.tile_pool(name="ws", bufs=1))
            st = stack.enter_context(tc.tile_pool(name="st", bufs=2))
            psmm = stack.enter_context(
                tc.tile_pool(name="psmm", bufs=2, space="PSUM"))

            # ---------------- constants ----------------
            ident = consts.tile([128, 128], F32)
            nc.sync.dma_start(ident[:], ident_d[:])
            identr = consts.tile([128, 128], F32R)
            nc.sync.dma_start(identr[:], identr_d[:])
            onesr = consts.tile([1, 128], F32R)
            nc.sync.dma_start(onesr[:], onesr_d[:])
            ones32 = consts.tile([1, 128], F32)
            nc.sync.dma_start(ones32[:], ones32_d[:])
            hones = consts.tile([128, 4, H], BF16)
            nc.sync.dma_start(hones[:], hones_d[:])
            maskw = consts.tile([128, 4, NM], F32R)
            nc.sync.dma_start(maskw[:], maskw_d.rearrange("(c p) n -> p c n", p=128))
            maskb = consts.tile([NM, 1], F32)
            nc.sync.dma_start(maskb[:], maskb_d[:])
            onescol_bf = consts.tile([128, 1], BF16)
            nc.gpsimd.memset(onescol_bf[:], 1.0)
            lneps = consts.tile([128, 1], F32)
            nc.gpsimd.memset(lneps[:], 1e-5)

            # persistent activations: x[b] = (128 t-part, 4 t-chunks, DIM)
            xs = [xpool.tile([128, 4, DIM], F32, name=f"x{b}") for b in range(n_b)]

            # ---------------- embedding ----------------
            # (reuses later-phase slots: embw->wv, pos->o_all, mel->hT)
            embw = wpool.tile([128, 2, DIM], F32R, tag="wv", name="embw")
            nc.sync.dma_start(embw[:], embw_d.rearrange("(c p) d -> p c d", p=128))
            pos = ws.tile([128, 4, DIM], F32, tag="o_all", bufs=1, name="pos")
            nc.sync.dma_start(pos[:], pos_d.rearrange("(c p) d -> p c d", p=128))
            for b in range(n_b):
                mel_sb = ws.tile([128, 2, T], F32R, tag="hT", bufs=2,
                                 name=f"mel{b}")
                nc.sync.dma_start(
                    mel_sb[:], mel_d[b].rearrange("(c p) t -> p c t", p=128))
                for tcn in range(4):
                    ps = psmm.tile([128, DIM], F32, tag="mm",
                                   name=f"emb{nc.next_id()}")
                    for k in range(2):
                        nc.tensor.matmul(
                            ps[:], mel_sb[:, k, tcn * 128:(tcn + 1) * 128],
                            embw[:, k], start=(k == 0), stop=(k == 1))
                    nc.vector.tensor_add(xs[b][:, tcn], ps[:], pos[:, tcn])

            # layer weights (single-buffered; loads overlap prior-layer compute)
            def load_layer_weights(l):
                wqk = wpool.tile([128, 4, 2 * DIM], F32R, tag="wqk", name=f"wqk{l}")
                nc.sync.dma_start(wqk[:], wqk_d[l].rearrange("(c p) n -> p c n", p=128))
                wv = wpool.tile([128, 4, DIM], F32R, tag="wv", name=f"wv{l}")
                nc.sync.dma_start(wv[:], wv_d[l].rearrange("(c p) n -> p c n", p=128))
                wtp = wpool.tile([128, M], BF16, tag="wtp", name=f"wtp{l}")
                nc.sync.dma_start(wtp[:], wtp_d[l])
                outw = wpool.tile([128, 4, DIM], F32R, tag="outw", name=f"outw{l}")
                nc.sync.dma_start(outw[:], outw_d[l].rearrange("(c p) n -> p c n", p=128))
                w1 = wpool.tile([128, 4, FFD], F32R, tag="w1", name=f"w1{l}")
                nc.sync.dma_start(w1[:], w1_d[l].rearrange("(c p) n -> p c n", p=128))
                w2 = wpool.tile([128, 16, DIM], F32R, tag="w2", name=f"w2{l}")
                nc.sync.dma_start(w2[:], w2_d[l].rearrange("(c p) n -> p c n", p=128))
                d = {"wqk": wqk, "wv": wv, "wtp": wtp, "outw": outw,
                     "w1": w1, "w2": w2}
                if ubqk:
                    bqk = wpool.tile([128, 8], F32, tag="bqk", name=f"bqk{l}")
                    nc.sync.dma_start(bqk[:], bqk_d[l].rearrange("(c p) -> p c", p=128))
                    d["bqk"] = bqk
                if ubv:
                    bv = wpool.tile([1, DIM], F32R, tag="bv", name=f"bv{l}")
                    nc.sync.dma_start(bv[:], bv_d[l])
                    d["bv"] = bv
                if ubo:
                    outb = wpool.tile([1, DIM], F32R, tag="outb", name=f"outb{l}")
                    nc.sync.dma_start(outb[:], outb_d[l])
                    d["outb"] = outb
                if ub1:
                    b1 = wpool.tile([128, 16], F32, tag="b1", name=f"b1{l}")
                    nc.sync.dma_start(b1[:], b1_d[l].rearrange("(c p) -> p c", p=128))
                    d["b1"] = b1
                if ub2:
                    b2 = wpool.tile([1, DIM], F32R, tag="b2", name=f"b2{l}")
                    nc.sync.dma_start(b2[:], b2_d[l])
                    d["b2"] = b2
                return d

            def layernorm_transposed(xb):
                """LN over the free dim of each (128,512) chunk of xb; output
                PE-transposed into a (128, 4, T) fp32r tile (D-major)."""
                hT = ws.tile([128, 4, T], F32R, tag="hT", bufs=2,
                             name=f"hT{nc.next_id()}")
                for tcn in range(4):
                    xtc = xb[:, tcn]
                    mv = st.tile([128, 2], F32, tag="mv", name=f"mv{nc.next_id()}")
                    if USE_BN:
                        bns = st.tile([128, 6], F32, tag="bns", name=f"bns{nc.next_id()}")
                        nc.vector.bn_stats(bns[:], xtc)
                        nc.vector.bn_aggr(mv[:], bns[:])
                    else:
                        dump = ws.tile([128, T], F32, tag="dump", bufs=1,
                                       name=f"dmp{nc.next_id()}")
                        ssq = st.tile([128, 1], F32, tag="ssq", name=f"sq{nc.next_id()}")
                        nc.vector.scalar_tensor_tensor(
                            dump[:], xtc, 1.0, xtc, op0=ALU.mult, op1=ALU.mult,
                            accum_out=ssq[:])
                        nc.vector.reduce_sum(mv[:, 0:1], xtc, axis=AXX)
                        nc.vector.tensor_scalar_mul(mv[:, 0:1], mv[:, 0:1], 1.0 / DIM)
                        musq = st.tile([128, 1], F32, tag="musq", name=f"mq{nc.next_id()}")
                        nc.scalar.square(musq[:], mv[:, 0:1])
                        nc.vector.scalar_tensor_tensor(
                            mv[:, 1:2], ssq[:], 1.0 / DIM, musq[:],
                            op0=ALU.mult, op1=ALU.subtract)
                    std = st.tile([128, 1], F32, tag="std", name=f"std{nc.next_id()}")
                    nc.scalar.activation(std[:], mv[:, 1:2], AFT.Sqrt,
                                         bias=lneps[:])
                    rstd = st.tile([128, 1], F32, tag="rstd", name=f"rs{nc.next_id()}")
                    nc.vector.reciprocal(rstd[:], std[:])
                    h = ws.tile([128, T], F32R, tag="h", bufs=2,
                                name=f"h{nc.next_id()}")
                    nc.vector.tensor_scalar(h[:], xtc, mv[:, 0:1], rstd[:],
                                            op0=ALU.subtract, op1=ALU.mult)
                    tp = psmm.tile([128, 4, 128], F32R, tag="mm",
                                   name=f"tp{nc.next_id()}")
                    for i in range(4):
                        nc.tensor.transpose(tp[:, i], h[:, i * 128:(i + 1) * 128],
                                            identr[:])
                    nc.vector.tensor_copy(
                        hT[:, :, tcn * 128:(tcn + 1) * 128], tp[:])
                return hT

            def transpose_o(o_all):
                oT = ws.tile([128, 4, T], F32R, tag="oT", bufs=1,
                             name=f"oT{nc.next_id()}")
                for tcn in range(4):
                    tp = psmm.tile([128, 4, 128], F32R, tag="mm",
                                   name=f"otp{nc.next_id()}")
                    for i in range(4):
                        nc.tensor.transpose(
                            tp[:, i], o_all[:, tcn, i * 128:(i + 1) * 128],
                            identr[:])
                    nc.vector.tensor_copy(
                        oT[:, :, tcn * 128:(tcn + 1) * 128], tp[:])
                return oT

            def attention(l, wts, b, psatt):
                hT = layernorm_transposed(xs[b])
                wqk, wv, wtp = wts["wqk"], wts["wv"], wts["wtp"]

                # q^T,k^T feature-major in bf16: qkT[:, fc] = (128 feat, T)
                qkT = ws.tile([128, 8, T], BF16, tag="qkT", bufs=1,
                              name=f"qkT{nc.next_id()}")
                for fc in range(8):
                    ps = psmm.tile([128, T], F32, tag="mm", name=f"qk{nc.next_id()}")
                    for k in range(4):
                        nc.tensor.matmul(
                            ps[:], wqk[:, k, fc * 128:(fc + 1) * 128], hT[:, k],
                            start=(k == 0), stop=(k == 3))
                    if ubqk:
                        nc.scalar.activation(qkT[:, fc], ps[:], AFT.Identity,
                                             bias=wts["bqk"][:, fc:fc + 1])
                    else:
                        nc.scalar.copy(qkT[:, fc], ps[:])

                # v (t-major) into strided bf16 vx with ones columns
                vx = ws.tile([128, 4, H, 65], BF16, tag="vx", bufs=1,
                             name=f"vx{nc.next_id()}")
                for tcn in range(4):
                    nc.vector.memset(vx[:, tcn, :, 64:65], 1.0)
                    ps = psmm.tile([128, DIM], F32, tag="mm", name=f"v{nc.next_id()}")
                    for k in range(4):
                        nc.tensor.matmul(
                            ps[:], hT[:, k, tcn * 128:(tcn + 1) * 128], wv[:, k],
                            start=(k == 0), stop=(k == 3 and not ubv))
                    if ubv:
                        nc.tensor.matmul(ps[:], onesr[:], wts["bv"][:],
                                         start=False, stop=True)
                    nc.vector.tensor_copy(
                        vx[:, tcn, :, 0:64],
                        ps.rearrange("p (h d) -> p h d", d=64))

                # q_sq/k_sq: bf16 squares + blockdiag-halfones matmuls
                # qksq[:, tc, 0:8] = 0.5*sum qd^2 per head ; [:, tc, 8:16] = k
                qksq = ws.tile([128, 4, 16], F32, tag="qksq", bufs=1,
                               name=f"qksq{nc.next_id()}")
                for half in range(2):          # 0: q (chunks 0-3), 1: k (4-7)
                    sqs = []
                    for k in range(4):
                        sq = ws.tile([128, T], BF16, tag="sq", bufs=4,
                                     name=f"sq{nc.next_id()}")
                        qk = qkT[:, 4 * half + k]
                        nc.vector.scalar_tensor_tensor(
                            sq[:], qk, 1.0, qk, op0=ALU.mult, op1=ALU.mult)
                        sqs.append(sq)
                    for tcn in range(4):
                        pst = psatt.tile([128, H], F32, tag="tiny",
                                         name=f"sqp{nc.next_id()}")
                        for k in range(4):
                            nc.tensor.matmul(
                                pst[:], sqs[k][:, tcn * 128:(tcn + 1) * 128],
                                hones[:, k], start=(k == 0), stop=(k == 3))
                        nc.scalar.copy(qksq[:, tcn, 8 * half:8 * half + 8], pst[:])

                o_all = ws.tile([128, 4, DIM], F32R, tag="o_all", bufs=1,
                                name=f"o{nc.next_id()}")

                # Heads processed in even/odd pairs: the K=64 random-feature
                # matmuls of the two heads run concurrently in the PE array
                # via row tiling (partitions 0-63 / 64-127 of the same qkT
                # d-chunk; wtp rows are host-duplicated).
                for dc in range(4):
                    heads = (2 * dc, 2 * dc + 1)
                    kc = 4 + dc
                    PR = ((0, 64, None), (64, 128, (64, 0))) if USE_PAIR \
                        else ((0, 64, None), (64, 128, (64, 0)))

                    # --- pass 1: kp for kmax only (psum freed immediately)
                    kmxc = [st.tile([128, 4], F32, tag=f"kmxc{i}",
                                    name=f"kmc{nc.next_id()}") for i in range(2)]
                    for tcn in range(4):
                        kp2 = [psatt.tile([128, M], F32, tag="h256", bufs=4,
                                          name=f"kp2{nc.next_id()}")
                               for _ in range(2)]
                        for i, (o0, o1, tpos) in enumerate(PR):
                            nc.tensor.matmul(
                                kp2[i][:], qkT[o0:o1, kc, tcn * 128:(tcn + 1) * 128],
                                wtp[o0:o1], start=True, stop=True,
                                tile_position=tpos)
                        for i in range(2):
                            nc.vector.reduce_max(kmxc[i][:, tcn:tcn + 1],
                                                 kp2[i][:], axis=AXX)
                    kbc = []
                    for i in range(2):
                        km1 = st.tile([128, 1], F32, tag=f"km1{i}",
                                      name=f"km1{nc.next_id()}")
                        nc.vector.reduce_max(km1[:], kmxc[i][:], axis=AXX)
                        kb_bc = st.tile([128, 1], F32, tag=f"kbc{i}",
                                        name=f"kbc{nc.next_id()}")
                        if USE_PAR:
                            nc.gpsimd.partition_all_reduce(
                                kb_bc[:], km1[:], 128, bass_isa.ReduceOp.max)
                        else:
                            kmt = psatt.tile([1, 128], F32, tag="tiny",
                                             name=f"kmt{nc.next_id()}")
                            nc.tensor.transpose(kmt[:], km1[:], ident[:])
                            kms = st.tile([1, 1], F32, tag=f"kms{i}",
                                          name=f"kms{nc.next_id()}")
                            nc.vector.reduce_max(kms[:], kmt[:], axis=AXX)
                            kbc_ps = psatt.tile([128, 1], F32, tag="tiny",
                                                name=f"kbp{nc.next_id()}")
                            nc.tensor.matmul(kbc_ps[:], ones32[:], kms[:],
                                             start=True, stop=True)
                            nc.scalar.copy(kb_bc[:], kbc_ps[:])
                        kbc.append(kb_bc)

                    # --- qmax (per query) for both heads
                    qmaxs = [st.tile([128, 4], F32, tag=f"qmx{i}",
                                     name=f"qmx{nc.next_id()}") for i in range(2)]
                    for tcn in range(4):
                        qp2 = [psatt.tile([128, M], F32, tag="h256", bufs=4,
                                          name=f"qp2{nc.next_id()}")
                               for _ in range(2)]
                        for i, (o0, o1, tpos) in enumerate(PR):
                            nc.tensor.matmul(
                                qp2[i][:], qkT[o0:o1, dc, tcn * 128:(tcn + 1) * 128],
                                wtp[o0:o1], start=True, stop=True,
                                tile_position=tpos)
                        for i in range(2):
                            nc.vector.reduce_max(qmaxs[i][:, tcn:tcn + 1],
                                                 qp2[i][:], axis=AXX)

                    # --- pass 2: kp again -> k_phi = exp(kp-ksq-kmax)+EPS
                    kphis = []
                    for i, (o0, o1, tpos) in enumerate(PR):
                        h = heads[i]
                        kb4 = st.tile([128, 4], F32, tag=f"kb4{i}",
                                      name=f"kb4{nc.next_id()}")
                        nc.vector.tensor_scalar(
                            kb4[:], qksq[:, :, 8 + h], -1.0, kbc[i][:],
                            op0=ALU.mult, op1=ALU.subtract)
                        kphis.append((h, kb4))
                    kphi2 = [ws.tile([128, 4, M], BF16, tag=f"kphi{i}", bufs=1,
                                     name=f"kph{nc.next_id()}") for i in range(2)]
                    for tcn in range(4):
                        kp2 = [psatt.tile([128, M], F32, tag="h256", bufs=4,
                                          name=f"kpb{nc.next_id()}")
                               for _ in range(2)]
                        for i, (o0, o1, tpos) in enumerate(PR):
                            nc.tensor.matmul(
                                kp2[i][:], qkT[o0:o1, kc, tcn * 128:(tcn + 1) * 128],
                                wtp[o0:o1], start=True, stop=True,
                                tile_position=tpos)
                        for i in range(2):
                            nc.scalar.activation(kphi2[i][:, tcn], kp2[i][:],
                                                 AFT.Exp, bias=kphis[i][1][:, tcn:tcn + 1])
                    for i in range(2):
                        nc.vector.tensor_scalar_add(
                            kphi2[i].rearrange("p c m -> p (c m)"),
                            kphi2[i].rearrange("p c m -> p (c m)"), EPS)

                    # --- e^{qp^T} (M-major) for both heads
                    eqpT2 = [ws.tile([128, 2, T], BF16, tag=f"eqpT{i}", bufs=1,
                                     name=f"eq{nc.next_id()}") for i in range(2)]
                    for mh in range(2):
                        pss = []
                        for i, (o0, o1, tpos) in enumerate(PR):
                            ps = psmm.tile([128, T], F32, tag="mm",
                                           name=f"qpT{nc.next_id()}")
                            nc.tensor.matmul(
                                ps[:], wtp[o0:o1, mh * 128:(mh + 1) * 128],
                                qkT[o0:o1, dc], start=True, stop=True,
                                tile_position=tpos)
                            pss.append(ps)
                        for i in range(2):
                            nc.scalar.activation(eqpT2[i][:, mh], pss[i][:],
                                                 AFT.Exp)

                    # --- per-head tail: kvx, row_s bcast, A, divide
                    for i in range(2):
                        h = heads[i]
                        kphi = kphi2[i]
                        eqpT = eqpT2[i]

                        kvx_ps = psatt.tile([128, 2, 65], F32, tag="tiny",
                                            name=f"kvp{nc.next_id()}")
                        for mh in range(2):
                            for tcn in range(4):
                                nc.tensor.matmul(
                                    kvx_ps[:, mh],
                                    kphi[:, tcn, mh * 128:(mh + 1) * 128],
                                    vx[:, tcn, h], start=(tcn == 0),
                                    stop=(tcn == 3))
                        kvx = ws.tile([128, 2, 65], BF16, tag="kvx", bufs=2,
                                      name=f"kvs{nc.next_id()}")
                        nc.scalar.copy(kvx[:], kvx_ps[:])

                        rs_ps = psatt.tile([1, 65], F32, tag="tiny",
                                           name=f"rsp{nc.next_id()}")
                        for mh in range(2):
                            nc.tensor.matmul(rs_ps[:], onescol_bf[:], kvx[:, mh],
                                             start=(mh == 0), stop=(mh == 1))
                        rs = st.tile([1, 65], F32, tag="rs",
                                     name=f"rss{nc.next_id()}")
                        nc.scalar.mul(rs[:], rs_ps[:], EPS)
                        nc.vector.tensor_scalar_add(rs[0:1, 64:65],
                                                    rs[0:1, 64:65], DEN_EPS)
                        sb_ps = psatt.tile([128, 65], F32, tag="tiny",
                                           name=f"sbp{nc.next_id()}")
                        nc.tensor.matmul(sb_ps[:], ones32[:], rs[:],
                                         start=True, stop=True)
                        s_b = st.tile([128, 65], F32, tag="s_b",
                                      name=f"sbb{nc.next_id()}")
                        nc.scalar.copy(s_b[:], sb_ps[:])

                        A_ps = psatt.tile([128, 4, 65], F32, tag="tiny",
                                          name=f"A{nc.next_id()}")
                        for tcn in range(4):
                            for mh in range(2):
                                nc.tensor.matmul(
                                    A_ps[:, tcn],
                                    eqpT[:, mh, tcn * 128:(tcn + 1) * 128],
                                    kvx[:, mh], start=(mh == 0), stop=(mh == 1))

                        gsum = st.tile([128, 4], F32, tag="gsum",
                                       name=f"gs{nc.next_id()}")
                        nc.vector.tensor_add(gsum[:], qksq[:, :, h], qmaxs[i][:])
                        gam = st.tile([128, 4], F32, tag="gam",
                                      name=f"gam{nc.next_id()}")
                        nc.scalar.activation(gam[:], gsum[:], AFT.Exp)
                        oe4 = st.tile([128, 4, 65], F32, tag="oe",
                                      name=f"oe{nc.next_id()}")
                        for tcn in range(4):
                            nc.vector.scalar_tensor_tensor(
                                oe4[:, tcn], s_b[:], gam[:, tcn:tcn + 1],
                                A_ps[:, tcn], op0=ALU.mult, op1=ALU.add)
                        rec4 = st.tile([128, 4], F32, tag="rec",
                                       name=f"rc{nc.next_id()}")
                        nc.vector.reciprocal(rec4[:], oe4[:, :, 64])
                        for tcn in range(4):
                            nc.vector.tensor_scalar_mul(
                                o_all[:, tcn, h * 64:(h + 1) * 64],
                                oe4[:, tcn, 0:64], rec4[:, tcn:tcn + 1])

                # out-proj + residual
                oT = transpose_o(o_all)
                outw = wts["outw"]
                for tcn in range(4):
                    ps = psmm.tile([128, DIM], F32, tag="mm",
                                   name=f"op{nc.next_id()}")
                    for k in range(4):
                        nc.tensor.matmul(
                            ps[:], oT[:, k, tcn * 128:(tcn + 1) * 128],
                            outw[:, k], start=(k == 0),
                            stop=(k == 3 and not ubo))
                    if ubo:
                        nc.tensor.matmul(ps[:], onesr[:], wts["outb"][:],
                                         start=False, stop=True)
                    nc.vector.tensor_add(xs[b][:, tcn], ps[:], xs[b][:, tcn])

            def ffn(l, wts, b, psffn):
                h2T = layernorm_transposed(xs[b])
                w1, w2 = wts["w1"], wts["w2"]
                accs = [psffn.tile([128, DIM], F32, tag="acc",
                                   name=f"fa{nc.next_id()}") for _ in range(4)]
                for fc in range(16):
                    ps = psmm.tile([128, T], F32, tag="mm",
                                   name=f"g1{nc.next_id()}")
                    for k in range(4):
                        nc.tensor.matmul(
                            ps[:], w1[:, k, fc * 128:(fc + 1) * 128], h2T[:, k],
                            start=(k == 0), stop=(k == 3))
                    gt = ws.tile([128, T], F32R, tag="gt", bufs=2,
                                 name=f"gt{nc.next_id()}")
                    if ub1:
                        nc.scalar.activation(gt[:], ps[:], AFT.Gelu_apprx_tanh,
                                             bias=wts["b1"][:, fc:fc + 1])
                    else:
                        nc.scalar.activation(gt[:], ps[:], AFT.Gelu_apprx_tanh)
                    for tcn in range(4):
                        nc.tensor.matmul(
                            accs[tcn][:], gt[:, tcn * 128:(tcn + 1) * 128],
                            w2[:, fc], start=(fc == 0),
                            stop=(fc == 15 and not ub2))
                for tcn in range(4):
                    if ub2:
                        nc.tensor.matmul(accs[tcn][:], onesr[:], wts["b2"][:],
                                         start=False, stop=True)
                    nc.vector.tensor_add(xs[b][:, tcn], accs[tcn][:],
                                         xs[b][:, tcn])

            # ---------------- layers ----------------
            for l in range(n_layers):
                wts = load_layer_weights(l)
                with tc.tile_pool(name=f"psatt{l}", bufs=2,
                                  space="PSUM") as psatt:
                    for b in range(n_b):
                        attention(l, wts, b, psatt)
                with tc.tile_pool(name=f"psffn{l}", bufs=4,
                                  space="PSUM") as psffn:
                    for b in range(n_b):
                        ffn(l, wts, b, psffn)

            # ---------------- final masks ----------------
            for b in range(n_b):
                xT = ws.tile([128, 4, T], F32R, tag="hT", bufs=2,
                             name=f"xT{nc.next_id()}")
                for tcn in range(4):
                    tp = psmm.tile([128, 4, 128], F32, tag="mm",
                                   name=f"xtp{nc.next_id()}")
                    for i in range(4):
                        nc.tensor.transpose(
                            tp[:, i], xs[b][:, tcn, i * 128:(i + 1) * 128],
                            ident[:])
                    nc.vector.tensor_copy(
                        xT[:, :, tcn * 128:(tcn + 1) * 128], tp[:])
                yps = psmm.tile([128, T], F32, tag="mm", name=f"y{nc.next_id()}")
                for k in range(4):
                    nc.tensor.matmul(yps[0:NM], maskw[:, k], xT[:, k],
                                     start=(k == 0), stop=(k == 3))
                ysb = ws.tile([NM, T], F32, tag="ysb", bufs=1,
                              name=f"ys{nc.next_id()}")
                if ubm:
                    nc.scalar.activation(ysb[:], yps[0:NM], AFT.Sigmoid,
                                         bias=maskb[:])
                else:
                    nc.scalar.activation(ysb[:], yps[0:NM], AFT.Sigmoid)
                nc.sync.dma_start(out_d[b], ysb[:])

    nc.compile()
    return nc


def _prep_inputs(inputs, n_layers=L, n_b_total=B):
    """Host-side weight folding/rounding. Returns (per-core in_maps, flags)."""
    import ml_dtypes
    bf16 = ml_dtypes.bfloat16
    f32 = lambda a: np.ascontiguousarray(a, np.float32)
    mel = f32(inputs["mel"])[:n_b_total]
    to_emb_w = f32(inputs["to_emb_w"])
    to_emb_b = f32(inputs["to_emb_b"])
    pos_emb = f32(inputs["pos_emb"])
    proj = f32(inputs["proj"])
    qkv_w = f32(inputs["qkv_w"])
    qkv_b = f32(inputs["qkv_b"])
    out_w = f32(inputs["out_w"])
    out_b = f32(inputs["out_b"])
    ln1_g = f32(inputs["ln1_g"])
    ln1_b = f32(inputs["ln1_b"])
    ln2_g = f32(inputs["ln2_g"])
    ln2_b = f32(inputs["ln2_b"])
    ff1_w = f32(inputs["ff1_w"])
    ff1_b = f32(inputs["ff1_b"])
    ff2_w = f32(inputs["ff2_w"])
    ff2_b = f32(inputs["ff2_b"])
    mask_w = f32(inputs["mask_w"])
    mask_b = f32(inputs["mask_b"])

    nl = n_layers
    Wfold = qkv_w[:nl] * ln1_g[:nl][:, :, None]          # (L, D, 3D)
    bias_qkv = np.einsum("ld,ldn->ln", ln1_b[:nl], qkv_w[:nl]) + qkv_b[:nl]
    wq = Wfold[:, :, :DIM] * DN
    wk = Wfold[:, :, DIM:2 * DIM] * DN
    wv = Wfold[:, :, 2 * DIM:]
    bqk = np.concatenate([bias_qkv[:, :DIM] * DN,
                          bias_qkv[:, DIM:2 * DIM] * DN], axis=1)  # (L, 1024)
    bv = bias_qkv[:, None, 2 * DIM:]                     # (L, 1, D)
    W1fold = ff1_w[:nl] * ln2_g[:nl][:, :, None]
    b1 = np.einsum("ld,ldn->ln", ln2_b[:nl], ff1_w[:nl]) + ff1_b[:nl]
    wtpT = np.transpose(proj[:nl], (0, 2, 1))            # (L, DH, M)
    wtp = np.concatenate([wtpT, wtpT], axis=1)           # (L, 128, M) doubled

    hones = np.zeros((128, 4, H), np.float32)
    for d in range(DIM):
        hones[d % 128, d // 128, d // DH] = 0.5
    ident = np.eye(128, dtype=np.float32)

    common = {
        "pos": f32(pos_emb[0, :T] + to_emb_b),
        "embw": _round_tf32(to_emb_w),
        "wqk": _round_tf32(np.concatenate([wq, wk], axis=2)),
        "bqk": f32(bqk),
        "wv": _round_tf32(wv),
        "bv": _round_tf32(bv),
        "wtp": np.ascontiguousarray(wtp.astype(bf16)),
        "outw": _round_tf32(out_w[:nl]),
        "outb": _round_tf32(out_b[:nl][:, None, :]),
        "w1": _round_tf32(W1fold),
        "b1": f32(b1),
        "w2": _round_tf32(ff2_w[:nl]),
        "b2": _round_tf32(ff2_b[:nl][:, None, :]),
        "maskw": _round_tf32(mask_w),
        "maskb": f32(mask_b[:, None]),
        "hones": np.ascontiguousarray(hones.astype(bf16)),
        "ident": ident,
        "identr": ident.copy(),
        "onesr": np.ones((1, 128), np.float32),
        "ones32": np.ones((1, 128), np.float32),
    }
    flags = (bool(np.any(bqk)), bool(np.any(bv)),
             bool(np.any(out_b[:nl])), bool(np.any(b1)),
             bool(np.any(ff2_b[:nl])), bool(np.any(mask_b)))

    mel_r = _round_tf32(mel)
    n_cores_used = max(1, n_b_total // BL)
    in_maps = []
    for c in range(n_cores_used):
        m = dict(common)
        m["mel"] = mel_r[c * BL:(c + 1) * BL]
        in_maps.append(m)
    return in_maps, flags


def kernel(**inputs):
    from concourse.bass_utils import run_bass_kernel_spmd

    in_maps, flags = _prep_inputs(inputs)
    key = ("full", flags)
    if key not in _CACHE:
        _CACHE[key] = _build(flags)
    nc = _CACHE[key]
    res = run_bass_kernel_spmd(nc, in_maps, list(range(NCORES)))
    out = np.concatenate([res.results[c]["masks"] for c in range(NCORES)],
                         axis=0)
    return np.ascontiguousarray(out, np.float32)


# revision 15
# speedup vs baseline: 1.3056x; 1.0332x over previous
"""Trainium2 Bass kernel for nn_PerformerSeperator (FAVOR+ transformer encoder).

Sharding: pure data-parallel over batch. B=32 is split 4-per-core across the
8 NeuronCores; every core runs the full 6-layer encoder on its shard with
replicated weights, so no collectives are needed.

Numerics: large matmuls run in fp32r (tf32; weights pre-rounded on host,
activations rounded by the producing ACT/DVE op). The per-head attention
matmuls (random-feature projections and the N=65 kvx/A contractions) run in
bf16. Everything else (layernorm, residual stream, FAVOR+ stabilizer
algebra) stays fp32. The eps/stabilizer algebra is restructured to be
layout-friendly while remaining exactly equivalent to the reference formula
(verified offline: fp32 impl matches reference to ~1e-7; tf32/bf16 rounding
contributes ~2e-4 absmax end to end).

Layouts: activations x live in SBUF as (T=4x128 partitions, DIM free) fp32
for the whole kernel. LN outputs are PE-transposed to D-major; q,k are
produced feature-major so per-head slices are partition ranges; v stays
t-major. The per-query stabilizer exp(q_sq + qmax) is applied as a
per-partition scalar in (T,.) layout against a partition-broadcast of the
eps * colsum(kvx) row, which keeps every op per-partition-scalar shaped.
"""
import os
import numpy as np

USE_PAR = os.environ.get("K_PAR", "1") == "1"
USE_PAIR = os.environ.get("K_PAIR", "1") == "1"
USE_BN = os.environ.get("K_BN", "1") == "1"

B, F, T = 32, 256, 512
DIM, L, H, M = 512, 6, 8, 256
DH = DIM // H            # 64
FFD = 4 * DIM            # 2048
NM = 4
NCORES = 8
BL = B // NCORES         # 4 batch elements per core
DN = DH ** -0.25
EPS = 1e-4
DEN_EPS = float(1e-6 * M)   # 1e-6 / ratio^2, ratio = M**-0.5

_CACHE = {}


def _round_tf32(x):
    """Round fp32 array to tf32 (10-bit mantissa, RNE). Matches PE fp32r."""
    x = np.ascontiguousarray(x, np.float32)
    u = x.view(np.uint32).astype(np.uint64)
    bias = ((u >> 13) & 1) + 0xFFF
    u = (u + bias) & ~np.uint64(0x1FFF)
    return u.astype(np.uint32).view(np.float32)


def _build(flags, n_layers=L, n_b=BL):
    """Build the per-core Bass program. flags = (ubqk, ubv, ubo, ub1, ub2, ubm)."""
    import contextlib
    import concourse.bacc as bacc
    import concourse.tile as tile
    from concourse import bass_isa, mybir

    ubqk, ubv, ubo, ub1, ub2, ubm = flags
    DT = mybir.dt
    AFT = mybir.ActivationFunctionType
    ALU = mybir.AluOpType
    AXX = mybir.AxisListType.X
    F32, F32R, BF16 = DT.float32, DT.float32r, DT.bfloat16

    nc = bacc.Bacc("TRN2", target_bir_lowering=False, debug=False,
                   num_devices=NCORES)

    # ---------------- DRAM I/O ----------------
    mel_d = nc.dram_tensor("mel", [n_b, F, T], F32R, kind="ExternalInput").ap()
    pos_d = nc.dram_tensor("pos", [T, DIM], F32, kind="ExternalInput").ap()
    embw_d = nc.dram_tensor("embw", [F, DIM], F32R, kind="ExternalInput").ap()
    wqk_d = nc.dram_tensor("wqk", [n_layers, DIM, 2 * DIM], F32R, kind="ExternalInput").ap()
    bqk_d = nc.dram_tensor("bqk", [n_layers, 2 * DIM], F32, kind="ExternalInput").ap()
    wv_d = nc.dram_tensor("wv", [n_layers, DIM, DIM], F32R, kind="ExternalInput").ap()
    bv_d = nc.dram_tensor("bv", [n_layers, 1, DIM], F32R, kind="ExternalInput").ap()
    wtp_d = nc.dram_tensor("wtp", [n_layers, 128, M], BF16, kind="ExternalInput").ap()
    outw_d = nc.dram_tensor("outw", [n_layers, DIM, DIM], F32R, kind="ExternalInput").ap()
    outb_d = nc.dram_tensor("outb", [n_layers, 1, DIM], F32R, kind="ExternalInput").ap()
    w1_d = nc.dram_tensor("w1", [n_layers, DIM, FFD], BF16, kind="ExternalInput").ap()
    b1_d = nc.dram_tensor("b1", [n_layers, FFD], F32, kind="ExternalInput").ap()
    w2_d = nc.dram_tensor("w2", [n_layers, FFD, DIM], BF16, kind="ExternalInput").ap()
    b2_d = nc.dram_tensor("b2", [n_layers, 1, DIM], BF16, kind="ExternalInput").ap()
    maskw_d = nc.dram_tensor("maskw", [DIM, NM], F32R, kind="ExternalInput").ap()
    maskb_d = nc.dram_tensor("maskb", [NM, 1], F32, kind="ExternalInput").ap()
    hones_d = nc.dram_tensor("hones", [128, 4, H], BF16, kind="ExternalInput").ap()
    ident_d = nc.dram_tensor("ident", [128, 128], F32, kind="ExternalInput").ap()
    identr_d = nc.dram_tensor("identr", [128, 128], F32R, kind="ExternalInput").ap()
    onesr_d = nc.dram_tensor("onesr", [1, 128], F32R, kind="ExternalInput").ap()
    ones32_d = nc.dram_tensor("ones32", [1, 128], F32, kind="ExternalInput").ap()
    out_d = nc.dram_tensor("masks", [n_b, NM, T], F32, kind="ExternalOutput").ap()

    with tile.TileContext(nc) as tc:
        with contextlib.ExitStack() as stack:
            consts = stack.enter_context(tc.tile_pool(name="consts", bufs=1))
            xpool = stack.enter_context(tc.tile_pool(name="xpool", bufs=1))
            wpool = stack.enter_context(tc.tile_pool(name="wpool", bufs=1))
            ws = stack.enter_context(tc.tile_pool(name="ws", bufs=1))
            st = stack.enter_context(tc.tile_pool(name="st", bufs=2))
            psmm = stack.enter_context(
                tc.tile_pool(name="psmm", bufs=2, space="PSUM"))

            # ---------------- constants ----------------
            ident = consts.tile([128, 128], F32)
            nc.sync.dma_start(ident[:], ident_d[:])
            identr = consts.tile([128, 128], F32R)
            nc.sync.dma_start(identr[:], identr_d[:])
            onesr = consts.tile([1, 128], F32R)
            nc.sync.dma_start(onesr[:], onesr_d[:])
            ones32 = consts.tile([1, 128], F32)
            nc.sync.dma_start(ones32[:], ones32_d[:])
            hones = consts.tile([128, 4, H], BF16)
            nc.sync.dma_start(hones[:], hones_d[:])
            maskw = consts.tile([128, 4, NM], F32R)
            nc.sync.dma_start(maskw[:], maskw_d.rearrange("(c p) n -> p c n", p=128))
            maskb = consts.tile([NM, 1], F32)
            nc.sync.dma_start(maskb[:], maskb_d[:])
            onescol_bf = consts.tile([128, 1], BF16)
            nc.gpsimd.memset(onescol_bf[:], 1.0)
            onesr_bf = consts.tile([1, 128], BF16)
            nc.gpsimd.memset(onesr_bf[:], 1.0)
            lneps = consts.tile([128, 1], F32)
            nc.gpsimd.memset(lneps[:], 1e-5)

            # persistent activations: x[b] = (128 t-part, 4 t-chunks, DIM)
            xs = [xpool.tile([128, 4, DIM], F32, name=f"x{b}") for b in range(n_b)]

            # ---------------- embedding ----------------
            # (reuses later-phase slots: embw->wv, pos->o_all, mel->hT)
            embw = wpool.tile([128, 2, DIM], F32R, tag="wv", name="embw")
            nc.sync.dma_start(embw[:], embw_d.rearrange("(c p) d -> p c d", p=128))
            pos = ws.tile([128, 4, DIM], F32, tag="o_all", bufs=1, name="pos")
            nc.sync.dma_start(pos[:], pos_d.rearrange("(c p) d -> p c d", p=128))
            for b in range(n_b):
                mel_sb = ws.tile([128, 2, T], F32R, tag="hT", bufs=2,
                                 name=f"mel{b}")
                nc.sync.dma_start(
                    mel_sb[:], mel_d[b].rearrange("(c p) t -> p c t", p=128))
                for tcn in range(4):
                    ps = psmm.tile([128, DIM], F32, tag="mm",
                                   name=f"emb{nc.next_id()}")
                    for k in range(2):
                        nc.tensor.matmul(
                            ps[:], mel_sb[:, k, tcn * 128:(tcn + 1) * 128],
                            embw[:, k], start=(k == 0), stop=(k == 1))
                    nc.vector.tensor_add(xs[b][:, tcn], ps[:], pos[:, tcn])

            # layer weights (single-buffered; loads overlap prior-layer compute)
            def load_layer_weights(l):
                wqk = wpool.tile([128, 4, 2 * DIM], F32R, tag="wqk", name=f"wqk{l}")
                nc.sync.dma_start(wqk[:], wqk_d[l].rearrange("(c p) n -> p c n", p=128))
                wv = wpool.tile([128, 4, DIM], F32R, tag="wv", name=f"wv{l}")
                nc.sync.dma_start(wv[:], wv_d[l].rearrange("(c p) n -> p c n", p=128))
                wtp = wpool.tile([128, M], BF16, tag="wtp", name=f"wtp{l}")
                nc.sync.dma_start(wtp[:], wtp_d[l])
                outw = wpool.tile([128, 4, DIM], F32R, tag="outw", name=f"outw{l}")
                nc.sync.dma_start(outw[:], outw_d[l].rearrange("(c p) n -> p c n", p=128))
                w1 = wpool.tile([128, 4, FFD], BF16, tag="w1", name=f"w1{l}")
                nc.sync.dma_start(w1[:], w1_d[l].rearrange("(c p) n -> p c n", p=128))
                w2 = wpool.tile([128, 16, DIM], BF16, tag="w2", name=f"w2{l}")
                nc.sync.dma_start(w2[:], w2_d[l].rearrange("(c p) n -> p c n", p=128))
                d = {"wqk": wqk, "wv": wv, "wtp": wtp, "outw": outw,
                     "w1": w1, "w2": w2}
                if ubqk:
                    bqk = wpool.tile([128, 8], F32, tag="bqk", name=f"bqk{l}")
                    nc.sync.dma_start(bqk[:], bqk_d[l].rearrange("(c p) -> p c", p=128))
                    d["bqk"] = bqk
                if ubv:
                    bv = wpool.tile([1, DIM], F32R, tag="bv", name=f"bv{l}")
                    nc.sync.dma_start(bv[:], bv_d[l])
                    d["bv"] = bv
                if ubo:
                    outb = wpool.tile([1, DIM], F32R, tag="outb", name=f"outb{l}")
                    nc.sync.dma_start(outb[:], outb_d[l])
                    d["outb"] = outb
                if ub1:
                    b1 = wpool.tile([128, 16], F32, tag="b1", name=f"b1{l}")
                    nc.sync.dma_start(b1[:], b1_d[l].rearrange("(c p) -> p c", p=128))
                    d["b1"] = b1
                if ub2:
                    b2 = wpool.tile([1, DIM], BF16, tag="b2", name=f"b2{l}")
                    nc.sync.dma_start(b2[:], b2_d[l])
                    d["b2"] = b2
                return d

            def layernorm_transposed(xb, out_dt):
                """LN over the free dim of each (128,512) chunk of xb; output
                PE-transposed into a (128, 4, T) tile (D-major)."""
                hT = ws.tile([128, 4, T], out_dt, tag="hT", bufs=2,
                             name=f"hT{nc.next_id()}")
                for tcn in range(4):
                    xtc = xb[:, tcn]
                    mv = st.tile([128, 2], F32, tag="mv", name=f"mv{nc.next_id()}")
                    if USE_BN:
                        bns = st.tile([128, 6], F32, tag="bns", name=f"bns{nc.next_id()}")
                        nc.vector.bn_stats(bns[:], xtc)
                        nc.vector.bn_aggr(mv[:], bns[:])
                    else:
                        dump = ws.tile([128, T], F32, tag="dump", bufs=1,
                                       name=f"dmp{nc.next_id()}")
                        ssq = st.tile([128, 1], F32, tag="ssq", name=f"sq{nc.next_id()}")
                        nc.vector.scalar_tensor_tensor(
                            dump[:], xtc, 1.0, xtc, op0=ALU.mult, op1=ALU.mult,
                            accum_out=ssq[:])
                        nc.vector.reduce_sum(mv[:, 0:1], xtc, axis=AXX)
                        nc.vector.tensor_scalar_mul(mv[:, 0:1], mv[:, 0:1], 1.0 / DIM)
                        musq = st.tile([128, 1], F32, tag="musq", name=f"mq{nc.next_id()}")
                        nc.scalar.square(musq[:], mv[:, 0:1])
                        nc.vector.scalar_tensor_tensor(
                            mv[:, 1:2], ssq[:], 1.0 / DIM, musq[:],
                            op0=ALU.mult, op1=ALU.subtract)
                    std = st.tile([128, 1], F32, tag="std", name=f"std{nc.next_id()}")
                    nc.scalar.activation(std[:], mv[:, 1:2], AFT.Sqrt,
                                         bias=lneps[:])
                    rstd = st.tile([128, 1], F32, tag="rstd", name=f"rs{nc.next_id()}")
                    nc.vector.reciprocal(rstd[:], std[:])
                    h = ws.tile([128, T], F32R, tag="h", bufs=2,
                                name=f"h{nc.next_id()}")
                    nc.vector.tensor_scalar(h[:], xtc, mv[:, 0:1], rstd[:],
                                            op0=ALU.subtract, op1=ALU.mult)
                    tp = psmm.tile([128, 4, 128], F32R, tag="mm",
                                   name=f"tp{nc.next_id()}")
                    for i in range(4):
                        nc.tensor.transpose(tp[:, i], h[:, i * 128:(i + 1) * 128],
                                            identr[:])
                    nc.scalar.copy(
                        hT[:, :, tcn * 128:(tcn + 1) * 128], tp[:])
                return hT

            def transpose_o(o_all):
                oT = ws.tile([128, 4, T], F32R, tag="oT", bufs=1,
                             name=f"oT{nc.next_id()}")
                for tcn in range(4):
                    tp = psmm.tile([128, 4, 128], F32R, tag="mm",
                                   name=f"otp{nc.next_id()}")
                    for i in range(4):
                        nc.tensor.transpose(
                            tp[:, i], o_all[:, tcn, i * 128:(i + 1) * 128],
                            identr[:])
                    nc.scalar.copy(
                        oT[:, :, tcn * 128:(tcn + 1) * 128], tp[:])
                return oT

            def attention(l, wts, b, psatt):
                hT = layernorm_transposed(xs[b], F32R)
                wqk, wv, wtp = wts["wqk"], wts["wv"], wts["wtp"]

                # q^T,k^T feature-major in bf16: qkT[:, fc] = (128 feat, T)
                qkT = ws.tile([128, 8, T], BF16, tag="qkT", bufs=2,
                              name=f"qkT{nc.next_id()}")
                for fc in range(8):
                    ps = psmm.tile([128, T], F32, tag="mm", name=f"qk{nc.next_id()}")
                    for k in range(4):
                        nc.tensor.matmul(
                            ps[:], wqk[:, k, fc * 128:(fc + 1) * 128], hT[:, k],
                            start=(k == 0), stop=(k == 3))
                    if ubqk:
                        nc.scalar.activation(qkT[:, fc], ps[:], AFT.Identity,
                                             bias=wts["bqk"][:, fc:fc + 1])
                    else:
                        nc.scalar.copy(qkT[:, fc], ps[:])

                # v (t-major) into strided bf16 vx with ones columns
                vx = ws.tile([128, 4, H, 65], BF16, tag="vx", bufs=1,
                             name=f"vx{nc.next_id()}")
                for tcn in range(4):
                    nc.vector.memset(vx[:, tcn, :, 64:65], 1.0)
                    ps = psmm.tile([128, DIM], F32, tag="mm", name=f"v{nc.next_id()}")
                    for k in range(4):
                        nc.tensor.matmul(
                            ps[:], hT[:, k, tcn * 128:(tcn + 1) * 128], wv[:, k],
                            start=(k == 0), stop=(k == 3 and not ubv))
                    if ubv:
                        nc.tensor.matmul(ps[:], onesr[:], wts["bv"][:],
                                         start=False, stop=True)
                    nc.vector.tensor_copy(
                        vx[:, tcn, :, 0:64],
                        ps.rearrange("p (h d) -> p h d", d=64))

                # q_sq/k_sq: bf16 squares + blockdiag-halfones matmuls
                # qksq[:, tc, 0:8] = 0.5*sum qd^2 per head ; [:, tc, 8:16] = k
                qksq = ws.tile([128, 4, 16], F32, tag="qksq", bufs=1,
                               name=f"qksq{nc.next_id()}")
                for half in range(2):          # 0: q (chunks 0-3), 1: k (4-7)
                    sqs = []
                    for k in range(4):
                        sq = ws.tile([128, T], BF16, tag="sq", bufs=4,
                                     name=f"sq{nc.next_id()}")
                        qk = qkT[:, 4 * half + k]
                        nc.vector.scalar_tensor_tensor(
                            sq[:], qk, 1.0, qk, op0=ALU.mult, op1=ALU.mult)
                        sqs.append(sq)
                    for tcn in range(4):
                        pst = psatt.tile([128, H], F32, tag="tiny",
                                         name=f"sqp{nc.next_id()}")
                        for k in range(4):
                            nc.tensor.matmul(
                                pst[:], sqs[k][:, tcn * 128:(tcn + 1) * 128],
                                hones[:, k], start=(k == 0), stop=(k == 3))
                        nc.scalar.copy(qksq[:, tcn, 8 * half:8 * half + 8], pst[:])

                o_all = ws.tile([128, 4, DIM], F32R, tag="o_all", bufs=1,
                                name=f"o{nc.next_id()}")

                # Heads processed in even/odd pairs: the K=64 random-feature
                # matmuls of the two heads run concurrently in the PE array
                # via row tiling (partitions 0-63 / 64-127 of the same qkT
                # d-chunk; wtp rows are host-duplicated).  The pair loop is
                # software-pipelined: pair j+1's matmul-heavy front overlaps
                # pair j's kvx/A/divide tail.
                def head_front(dc):
                    heads = (2 * dc, 2 * dc + 1)
                    kc = 4 + dc
                    PR = ((0, 64, None), (64, 128, (64, 0))) if USE_PAIR \
                        else ((0, 64, None), (64, 128, (64, 0)))

                    # --- pass 1: kp for kmax only (psum freed immediately)
                    kmxc = [st.tile([128, 4], F32, tag=f"kmxc{i}",
                                    name=f"kmc{nc.next_id()}") for i in range(2)]
                    for tcn in range(4):
                        kp2 = [psatt.tile([128, M], F32, tag="h256", bufs=4,
                                          name=f"kp2{nc.next_id()}")
                               for _ in range(2)]
                        for i, (o0, o1, tpos) in enumerate(PR):
                            nc.tensor.matmul(
                                kp2[i][:], qkT[o0:o1, kc, tcn * 128:(tcn + 1) * 128],
                                wtp[o0:o1], start=True, stop=True,
                                tile_position=tpos)
                        for i in range(2):
                            nc.vector.reduce_max(kmxc[i][:, tcn:tcn + 1],
                                                 kp2[i][:], axis=AXX)
                    kbc = []
                    for i in range(2):
                        km1 = st.tile([128, 1], F32, tag=f"km1{i}",
                                      name=f"km1{nc.next_id()}")
                        nc.vector.reduce_max(km1[:], kmxc[i][:], axis=AXX)
                        kb_bc = st.tile([128, 1], F32, tag=f"kbc{i}",
                                        name=f"kbc{nc.next_id()}")
                        if USE_PAR:
                            nc.gpsimd.partition_all_reduce(
                                kb_bc[:], km1[:], 128, bass_isa.ReduceOp.max)
                        else:
                            kmt = psatt.tile([1, 128], F32, tag="tiny",
                                             name=f"kmt{nc.next_id()}")
                            nc.tensor.transpose(kmt[:], km1[:], ident[:])
                            kms = st.tile([1, 1], F32, tag=f"kms{i}",
                                          name=f"kms{nc.next_id()}")
                            nc.vector.reduce_max(kms[:], kmt[:], axis=AXX)
                            kbc_ps = psatt.tile([128, 1], F32, tag="tiny",
                                                name=f"kbp{nc.next_id()}")
                            nc.tensor.matmul(kbc_ps[:], ones32[:], kms[:],
                                             start=True, stop=True)
                            nc.scalar.copy(kb_bc[:], kbc_ps[:])
                        kbc.append(kb_bc)

                    # --- qmax (per query) for both heads
                    qmaxs = [st.tile([128, 4], F32, tag=f"qmx{i}",
                                     name=f"qmx{nc.next_id()}") for i in range(2)]
                    for tcn in range(4):
                        qp2 = [psatt.tile([128, M], F32, tag="h256", bufs=4,
                                          name=f"qp2{nc.next_id()}")
                               for _ in range(2)]
                        for i, (o0, o1, tpos) in enumerate(PR):
                            nc.tensor.matmul(
                                qp2[i][:], qkT[o0:o1, dc, tcn * 128:(tcn + 1) * 128],
                                wtp[o0:o1], start=True, stop=True,
                                tile_position=tpos)
                        for i in range(2):
                            nc.vector.reduce_max(qmaxs[i][:, tcn:tcn + 1],
                                                 qp2[i][:], axis=AXX)

                    # --- pass 2: kp again -> k_phi = exp(kp-ksq-kmax)+EPS
                    kphis = []
                    for i, (o0, o1, tpos) in enumerate(PR):
                        h = heads[i]
                        kb4 = st.tile([128, 4], F32, tag=f"kb4{i}",
                                      name=f"kb4{nc.next_id()}")
                        nc.vector.tensor_scalar(
                            kb4[:], qksq[:, :, 8 + h], -1.0, kbc[i][:],
                            op0=ALU.mult, op1=ALU.subtract)
                        kphis.append((h, kb4))
                    kphi2 = [ws.tile([128, 4, M], BF16, tag=f"kphi{i}", bufs=2,
                                     name=f"kph{nc.next_id()}") for i in range(2)]
                    for tcn in range(4):
                        kp2 = [psatt.tile([128, M], F32, tag="h256", bufs=4,
                                          name=f"kpb{nc.next_id()}")
                               for _ in range(2)]
                        for i, (o0, o1, tpos) in enumerate(PR):
                            nc.tensor.matmul(
                                kp2[i][:], qkT[o0:o1, kc, tcn * 128:(tcn + 1) * 128],
                                wtp[o0:o1], start=True, stop=True,
                                tile_position=tpos)
                        for i in range(2):
                            nc.scalar.activation(kphi2[i][:, tcn], kp2[i][:],
                                                 AFT.Exp, bias=kphis[i][1][:, tcn:tcn + 1])
                    for i in range(2):
                        nc.vector.tensor_scalar_add(
                            kphi2[i].rearrange("p c m -> p (c m)"),
                            kphi2[i].rearrange("p c m -> p (c m)"), EPS)

                    # --- e^{qp^T} (M-major) for both heads
                    eqpT2 = [ws.tile([128, 2, T], BF16, tag=f"eqpT{i}", bufs=2,
                                     name=f"eq{nc.next_id()}") for i in range(2)]
                    for mh in range(2):
                        pss = []
                        for i, (o0, o1, tpos) in enumerate(PR):
                            ps = psmm.tile([128, T], F32, tag="mm",
                                           name=f"qpT{nc.next_id()}")
                            nc.tensor.matmul(
                                ps[:], wtp[o0:o1, mh * 128:(mh + 1) * 128],
                                qkT[o0:o1, dc], start=True, stop=True,
                                tile_position=tpos)
                            pss.append(ps)
                        for i in range(2):
                            nc.scalar.activation(eqpT2[i][:, mh], pss[i][:],
                                                 AFT.Exp)

                    return heads, kphi2, eqpT2, qmaxs

                def head_tail(state):
                    heads, kphi2, eqpT2, qmaxs = state
                    # per-head tail: kvx, row_s bcast, A, divide
                    for i in range(2):
                        h = heads[i]
                        kphi = kphi2[i]
                        eqpT = eqpT2[i]

                        kvx_ps = psatt.tile([128, 2, 65], F32, tag="tiny",
                                            name=f"kvp{nc.next_id()}")
                        for mh in range(2):
                            for tcn in range(4):
                                nc.tensor.matmul(
                                    kvx_ps[:, mh],
                                    kphi[:, tcn, mh * 128:(mh + 1) * 128],
                                    vx[:, tcn, h], start=(tcn == 0),
                                    stop=(tcn == 3))
                        kvx = ws.tile([128, 2, 65], BF16, tag="kvx", bufs=2,
                                      name=f"kvs{nc.next_id()}")
                        nc.scalar.copy(kvx[:], kvx_ps[:])

                        rs_ps = psatt.tile([1, 65], F32, tag="tiny",
                                           name=f"rsp{nc.next_id()}")
                        for mh in range(2):
                            nc.tensor.matmul(rs_ps[:], onescol_bf[:], kvx[:, mh],
                                             start=(mh == 0), stop=(mh == 1))
                        rs = st.tile([1, 65], F32, tag="rs",
                                     name=f"rss{nc.next_id()}")
                        nc.scalar.mul(rs[:], rs_ps[:], EPS)
                        nc.vector.tensor_scalar_add(rs[0:1, 64:65],
                                                    rs[0:1, 64:65], DEN_EPS)
                        sb_ps = psatt.tile([128, 65], F32, tag="tiny",
                                           name=f"sbp{nc.next_id()}")
                        nc.tensor.matmul(sb_ps[:], ones32[:], rs[:],
                                         start=True, stop=True)
                        s_b = st.tile([128, 65], F32, tag="s_b",
                                      name=f"sbb{nc.next_id()}")
                        nc.scalar.copy(s_b[:], sb_ps[:])

                        A_ps = psatt.tile([128, 4, 65], F32, tag="tiny",
                                          name=f"A{nc.next_id()}")
                        for tcn in range(4):
                            for mh in range(2):
                                nc.tensor.matmul(
                                    A_ps[:, tcn],
                                    eqpT[:, mh, tcn * 128:(tcn + 1) * 128],
                                    kvx[:, mh], start=(mh == 0), stop=(mh == 1))

                        gsum = st.tile([128, 4], F32, tag="gsum",
                                       name=f"gs{nc.next_id()}")
                        nc.vector.tensor_add(gsum[:], qksq[:, :, h], qmaxs[i][:])
                        gam = st.tile([128, 4], F32, tag="gam",
                                      name=f"gam{nc.next_id()}")
                        nc.scalar.activation(gam[:], gsum[:], AFT.Exp)
                        oe4 = st.tile([128, 4, 65], F32, tag="oe",
                                      name=f"oe{nc.next_id()}")
                        for tcn in range(4):
                            nc.vector.scalar_tensor_tensor(
                                oe4[:, tcn], s_b[:], gam[:, tcn:tcn + 1],
                                A_ps[:, tcn], op0=ALU.mult, op1=ALU.add)
                        rec4 = st.tile([128, 4], F32, tag="rec",
                                       name=f"rc{nc.next_id()}")
                        nc.vector.reciprocal(rec4[:], oe4[:, :, 64])
                        for tcn in range(4):
                            nc.vector.tensor_scalar_mul(
                                o_all[:, tcn, h * 64:(h + 1) * 64],
                                oe4[:, tcn, 0:64], rec4[:, tcn:tcn + 1])

                if os.environ.get("K_SWP", "1") == "1":
                    prev = None
                    for dc in range(4):
                        cur = head_front(dc)
                        if prev is not None:
                            head_tail(prev)
                        prev = cur
                    head_tail(prev)
                else:
                    for dc in range(4):
                        head_tail(head_front(dc))

                # out-proj + residual
                oT = transpose_o(o_all)
                outw = wts["outw"]
                for tcn in range(4):
                    ps = psmm.tile([128, DIM], F32, tag="mm",
                                   name=f"op{nc.next_id()}")
                    for k in range(4):
                        nc.tensor.matmul(
                            ps[:], oT[:, k, tcn * 128:(tcn + 1) * 128],
                            outw[:, k], start=(k == 0),
                            stop=(k == 3 and not ubo))
                    if ubo:
                        nc.tensor.matmul(ps[:], onesr[:], wts["outb"][:],
                                         start=False, stop=True)
                    nc.vector.tensor_add(xs[b][:, tcn], ps[:], xs[b][:, tcn])

            def ffn(l, wts, b, psffn):
                h2T = layernorm_transposed(xs[b], BF16)
                w1, w2 = wts["w1"], wts["w2"]
                accs = [psffn.tile([128, DIM], F32, tag="acc",
                                   name=f"fa{nc.next_id()}") for _ in range(4)]
                for fc in range(16):
                    ps = psmm.tile([128, T], F32, tag="mm",
                                   name=f"g1{nc.next_id()}")
                    for k in range(4):
                        nc.tensor.matmul(
                            ps[:], w1[:, k, fc * 128:(fc + 1) * 128], h2T[:, k],
                            start=(k == 0), stop=(k == 3))
                    gt = ws.tile([128, T], BF16, tag="gt", bufs=2,
                                 name=f"gt{nc.next_id()}")
                    if ub1:
                        nc.scalar.activation(gt[:], ps[:], AFT.Gelu_apprx_tanh,
                                             bias=wts["b1"][:, fc:fc + 1])
                    else:
                        nc.scalar.activation(gt[:], ps[:], AFT.Gelu_apprx_tanh)
                    for tcn in range(4):
                        nc.tensor.matmul(
                            accs[tcn][:], gt[:, tcn * 128:(tcn + 1) * 128],
                            w2[:, fc], start=(fc == 0),
                            stop=(fc == 15 and not ub2))
                for tcn in range(4):
                    if ub2:
                        nc.tensor.matmul(accs[tcn][:], onesr_bf[:], wts["b2"][:],
                                         start=False, stop=True)
                    nc.vector.tensor_add(xs[b][:, tcn], accs[tcn][:],
                                         xs[b][:, tcn])

            # ---------------- layers ----------------
            for l in range(n_layers):
                wts = load_layer_weights(l)
                with tc.tile_pool(name=f"psatt{l}", bufs=2,
                                  space="PSUM") as psatt:
                    for b in range(n_b):
                        attention(l, wts, b, psatt)
                with tc.tile_pool(name=f"psffn{l}", bufs=4,
                                  space="PSUM") as psffn:
                    for b in range(n_b):
                        ffn(l, wts, b, psffn)

            # ---------------- final masks ----------------
            for b in range(n_b):
                xT = ws.tile([128, 4, T], F32R, tag="hT", bufs=2,
                             name=f"xT{nc.next_id()}")
                for tcn in range(4):
                    tp = psmm.tile([128, 4, 128], F32, tag="mm",
                                   name=f"xtp{nc.next_id()}")
                    for i in range(4):
                        nc.tensor.transpose(
                            tp[:, i], xs[b][:, tcn, i * 128:(i + 1) * 128],
                            ident[:])
                    nc.scalar.copy(
                        xT[:, :, tcn * 128:(tcn + 1) * 128], tp[:])
                yps = psmm.tile([128, T], F32, tag="mm", name=f"y{nc.next_id()}")
                for k in range(4):
                    nc.tensor.matmul(yps[0:NM], maskw[:, k], xT[:, k],
                                     start=(k == 0), stop=(k == 3))
                ysb = ws.tile([NM, T], F32, tag="ysb", bufs=1,
                              name=f"ys{nc.next_id()}")
                if ubm:
                    nc.scalar.activation(ysb[:], yps[0:NM], AFT.Sigmoid,
                                         bias=maskb[:])
                else:
                    nc.scalar.activation(ysb[:], yps[0:NM], AFT.Sigmoid)
                nc.sync.dma_start(out_d[b], ysb[:])

    nc.compile()
    return nc


def _prep_inputs(inputs, n_layers=L, n_b_total=B):
    """Host-side weight folding/rounding. Returns (per-core in_maps, flags)."""
    import ml_dtypes
    bf16 = ml_dtypes.bfloat16
    f32 = lambda a: np.ascontiguousarray(a, np.float32)
    mel = f32(inputs["mel"])[:n_b_total]
    to_emb_w = f32(inputs["to_emb_w"])
    to_emb_b = f32(inputs["to_emb_b"])
    pos_emb = f32(inputs["pos_emb"])
    proj = f32(inputs["proj"])
    qkv_w = f32(inputs["qkv_w"])
    qkv_b = f32(inputs["qkv_b"])
    out_w = f32(inputs["out_w"])
    out_b = f32(inputs["out_b"])
    ln1_g = f32(inputs["ln1_g"])
    ln1_b = f32(inputs["ln1_b"])
    ln2_g = f32(inputs["ln2_g"])
    ln2_b = f32(inputs["ln2_b"])
    ff1_w = f32(inputs["ff1_w"])
    ff1_b = f32(inputs["ff1_b"])
    ff2_w = f32(inputs["ff2_w"])
    ff2_b = f32(inputs["ff2_b"])
    mask_w = f32(inputs["mask_w"])
    mask_b = f32(inputs["mask_b"])

    nl = n_layers
    Wfold = qkv_w[:nl] * ln1_g[:nl][:, :, None]          # (L, D, 3D)
    bias_qkv = np.einsum("ld,ldn->ln", ln1_b[:nl], qkv_w[:nl]) + qkv_b[:nl]
    wq = Wfold[:, :, :DIM] * DN
    wk = Wfold[:, :, DIM:2 * DIM] * DN
    wv = Wfold[:, :, 2 * DIM:]
    bqk = np.concatenate([bias_qkv[:, :DIM] * DN,
                          bias_qkv[:, DIM:2 * DIM] * DN], axis=1)  # (L, 1024)
    bv = bias_qkv[:, None, 2 * DIM:]                     # (L, 1, D)
    W1fold = ff1_w[:nl] * ln2_g[:nl][:, :, None]
    b1 = np.einsum("ld,ldn->ln", ln2_b[:nl], ff1_w[:nl]) + ff1_b[:nl]
    wtpT = np.transpose(proj[:nl], (0, 2, 1))            # (L, DH, M)
    wtp = np.concatenate([wtpT, wtpT], axis=1)           # (L, 128, M) doubled

    hones = np.zeros((128, 4, H), np.float32)
    for d in range(DIM):
        hones[d % 128, d // 128, d // DH] = 0.5
    ident = np.eye(128, dtype=np.float32)

    common = {
        "pos": f32(pos_emb[0, :T] + to_emb_b),
        "embw": _round_tf32(to_emb_w),
        "wqk": _round_tf32(np.concatenate([wq, wk], axis=2)),
        "bqk": f32(bqk),
        "wv": _round_tf32(wv),
        "bv": _round_tf32(bv),
        "wtp": np.ascontiguousarray(wtp.astype(bf16)),
        "outw": _round_tf32(out_w[:nl]),
        "outb": _round_tf32(out_b[:nl][:, None, :]),
        "w1": np.ascontiguousarray(W1fold.astype(bf16)),
        "b1": f32(b1),
        "w2": np.ascontiguousarray(ff2_w[:nl].astype(bf16)),
        "b2": np.ascontiguousarray(ff2_b[:nl][:, None, :].astype(bf16)),
        "maskw": _round_tf32(mask_w),
        "maskb": f32(mask_b[:, None]),
        "hones": np.ascontiguousarray(hones.astype(bf16)),
        "ident": ident,
        "identr": ident.copy(),
        "onesr": np.ones((1, 128), np.float32),
        "ones32": np.ones((1, 128), np.float32),
    }
    flags = (bool(np.any(bqk)), bool(np.any(bv)),
             bool(np.any(out_b[:nl])), bool(np.any(b1)),
             bool(np.any(ff2_b[:nl])), bool(np.any(mask_b)))

    mel_r = _round_tf32(mel)
    n_cores_used = max(1, n_b_total // BL)
    in_maps = []
    for c in range(n_cores_used):
        m = dict(common)
        m["mel"] = mel_r[c * BL:(c + 1) * BL]
        in_maps.append(m)
    return in_maps, flags


def kernel(**inputs):
    from concourse.bass_utils import run_bass_kernel_spmd

    in_maps, flags = _prep_inputs(inputs)
    key = ("full", flags)
    if key not in _CACHE:
        _CACHE[key] = _build(flags)
    nc = _CACHE[key]
    res = run_bass_kernel_spmd(nc, in_maps, list(range(NCORES)))
    out = np.concatenate([res.results[c]["masks"] for c in range(NCORES)],
                         axis=0)
    return np.ascontiguousarray(out, np.float32)
